# revision 1
# baseline (speedup 1.0000x reference)
"""Trainium2 Bass kernel for nn_ConstraintsModule (fuzzy-logic constraint
propagation).

Algorithm notes
---------------
The reference computes, twice (apply-1 with active=full_body, apply-2 with
active=unsat_head and goal-masked bodies):

    body_rev[b,c,a] = pb[c,a] + v[b,a]*(nb-pb)      -> max over a
    body_min[b,c]   = active[b,c] * (1 - max_a body_rev)
    lb[b,n] = max_c body_min * pos_head[c,n] ; ub = 1 - max_c body_min*neg_head
    u = max(min(lb,ub), min(max(lb,ub), v))

Because bodies are sparse (~4 literals/constraint) and heads are one-hot,
the dense [B, C, NA] tensor never needs to exist:

  max_a body_rev[b,c,:] = max over the constraint's literal list of
      v+[b,a] (pos literals) / v-[b,a] (neg literals)
  where apply-1: v+ = 1-p, v- = p ; apply-2: v+ = (1-g)(1-u1), v- = g*u1.

Sharding: constraints are owned by the core that owns their head atom
(atom range of 128 per core), so the head-scatter and clamp are core-local.
The device runs ONE compiled SPMD program twice (the two applies); the host
gathers per-literal value rows between launches (pure data layout) and
re-feeds them.  All reductions / matmuls / compares / clamps run on device:

  per core & launch:
    W[s,b]   = reduce_max over padded literal slots       (DVE)
    act[s,b] = (sum_a sgT[a,b]*lhsT[a,s] == target[s])    (PE bf16 + DVE cmp)
    bm       = act * (1 - W)                              (DVE)
    lb/ub    = one-hot scatter matmuls per collision layer (PE fp32, exact)
               + max across layers                        (DVE)
    u_slice  = max(min(lb,ub), min(max(lb,ub), base))     (DVE)
"""
import numpy as np

import concourse.bass as bass
import concourse.tile as tile
from concourse import mybir
from concourse.tile import ScopedClock
from concourse.bass_utils import run_bass_kernel_spmd

B = 128
NCOL = 2048
NA = 1024
C = 512
NCORES = 8
SLOTS = 128          # constraint slots per core (padded)
NLOC = 128           # atoms per core
KC = NA // 128       # contraction chunks for the active matmul


class FixedTileContext(tile.TileContext):
    """Two workarounds for this walrus/NRT combo: (1) skip the tail
    clear_and_free_semaphores — its InstSemClear makes NRT reject the NEFF at
    load, and NRT resets semaphores per execution anyway; (2) multi-wait
    instructions are split afterwards by split_multi_waits()."""

    def _drain_and_barrier(self, tick_clock, wait_clock):
        drain_inst = self.nc.sync.drain()
        wait_clock.add_sem_waits(
            drain_inst.ins, ScopedClock({None: tick_clock.global_clock})
        )
        self.nc.all_engine_barrier()
        assert self.sems is not None
        popped = self.nc._tile_sem_poison_stack.pop()
        assert popped is self._sem_poison
        self.nc.all_engine_barrier()


def split_multi_waits(nc: bass.Bass) -> int:
    """walrus here accepts only ONE sync wait per instruction; Tile's
    add_semaphores attaches several.  Hoist all but one wait onto fresh
    same-engine nops placed immediately before the instruction (engine
    program order is preserved, so blocking semantics are identical)."""
    n_split = 0
    for f in nc.m.functions:
        for b in f.blocks:
            new = []
            for ins in b.instructions:
                si = ins.sync_info
                waits = list(si.on_wait) if si and si.on_wait else []
                if len(waits) > 1:
                    for w in waits[:-1]:
                        nop = mybir.InstNoOp(
                            name=f"waitsplit-{n_split}", ins=[], outs=[])
                        n_split += 1
                        nop.engine = ins.engine
                        nop.sync_info = mybir.SyncInfo(on_wait=[w], on_update=[])
                        new.append(nop)
                    ins.sync_info = mybir.SyncInfo(
                        on_wait=[waits[-1]],
                        on_update=list(si.on_update) if si.on_update else [])
                new.append(ins)
            b.instructions = new
    return n_split


_PROGRAM_CACHE = {}
SPLIT_WAITS = True  # set False when running under CoreSim (sim chokes on the
                    # synthesized nops, and doesn't need the split anyway)


def _build_program(kpad: tuple, lpos: int, lneg: int) -> bass.Bass:
    """One SPMD apply phase.  Inputs are per-core; same program serves both
    applies (the lhsT / target / vperm / base inputs differ per launch).
    kpad = (k_hi, k_lo): slots are sorted by literal count, slots 0:64 use
    k_hi literal rows, slots 64:128 use k_lo."""
    key = (kpad, lpos, lneg)
    if key in _PROGRAM_CACHE:
        return _PROGRAM_CACHE[key]

    f32, bf16 = mybir.dt.float32, mybir.dt.bfloat16
    nc = bass.Bass(num_devices=NCORES)
    k_hi, k_lo = kpad
    vph_d = nc.declare_dram_parameter("vph", [64, k_hi * B], f32, isOutput=False)
    vpl_d = nc.declare_dram_parameter("vpl", [64, k_lo * B], f32, isOutput=False)
    # (two-group layout kept: slots sorted by literal count)
    sg_d = nc.declare_dram_parameter("sg", [128, KC * B], bf16, isOutput=False)
    lhsT_d = nc.declare_dram_parameter("lhsT", [128, KC * SLOTS], bf16, isOutput=False)
    targ_d = nc.declare_dram_parameter("targ", [SLOTS, 1], f32, isOutput=False)
    scat_d = nc.declare_dram_parameter(
        "scat", [SLOTS, (lpos + lneg) * NLOC], bf16, isOutput=False)
    base_d = nc.declare_dram_parameter("base", [NLOC, B], f32, isOutput=False)
    u_d = nc.declare_dram_parameter("u", [NLOC, B], f32, isOutput=True)

    with FixedTileContext(nc) as tc:
        with (
            tc.tile_pool(name="sbuf", bufs=1) as pool,
            tc.tile_pool(name="psum", bufs=1, space="PSUM") as psum,
        ):
            # Spread input loads across the two HWDGE rings (sync, scalar) and
            # SWDGE (gpsimd) so they don't serialize on one queue; PE-critical
            # tensors (lhsT, sg) go first on their ring.
            sg = pool.tile([128, KC, B], bf16)
            nc.sync.dma_start(sg[:], sg_d[:].rearrange("p (k b) -> p k b", k=KC))
            lh = pool.tile([128, KC, SLOTS], bf16)
            nc.sync.dma_start(lh[:], lhsT_d[:].rearrange("p (k s) -> p k s", k=KC))
            tg = pool.tile([SLOTS, 1], f32)
            nc.sync.dma_start(tg[:], targ_d[:])
            # vperm: slots sorted by literal count; the bottom 64 slots
            # need far fewer literal rows -> much smaller load + reduce
            vph = pool.tile([64, k_hi, B], f32)
            nc.scalar.dma_start(vph[:], vph_d[:].rearrange("s (k b) -> s k b", k=k_hi))
            vpl = pool.tile([64, k_lo, B], f32)
            nc.scalar.dma_start(vpl[:], vpl_d[:].rearrange("s (k b) -> s k b", k=k_lo))
            sc_b = pool.tile([SLOTS, lpos + lneg, NLOC], bf16)
            nc.sync.dma_start(
                sc_b[:], scat_d[:].rearrange("s (l n) -> s l n", l=lpos + lneg))
            sc = pool.tile([SLOTS, lpos + lneg, NLOC], f32)
            nc.scalar.copy(sc[:], sc_b[:])  # exact 0/1 upcast, off critical path
            bs = pool.tile([NLOC, B], f32)
            nc.scalar.dma_start(bs[:], base_d[:])

            # W[s,b] = max over literal slots (padding rows are 0.0)
            w = pool.tile([SLOTS, B], f32)
            nc.vector.tensor_reduce(
                out=w[:64, :], in_=vph[:].rearrange("s k b -> s b k"),
                axis=mybir.AxisListType.X, op=mybir.AluOpType.max)
            nc.vector.tensor_reduce(
                out=w[64:, :], in_=vpl[:].rearrange("s k b -> s b k"),
                axis=mybir.AxisListType.X, op=mybir.AluOpType.max)

            # act[s,b] = (sum_a lhsT[a,s]*sg[a,b] == targ[s])
            ps_act = psum.tile([SLOTS, B], f32)
            for k in range(KC):
                nc.tensor.matmul(
                    ps_act[:], lh[:, k, :], sg[:, k, :],
                    start=(k == 0), stop=(k == KC - 1))
            act = pool.tile([SLOTS, B], f32)
            nc.vector.tensor_scalar(
                act[:], ps_act[:], tg[:], None, mybir.AluOpType.is_equal)

            # bm = act * (1 - W)
            omw = pool.tile([SLOTS, B], f32)
            nc.vector.tensor_scalar(
                omw[:], w[:], -1.0, 1.0, mybir.AluOpType.mult, mybir.AluOpType.add)
            bm = pool.tile([SLOTS, B], f32)
            nc.vector.tensor_tensor(bm[:], act[:], omw[:], mybir.AluOpType.mult)

            # head scatter: lb = max over pos layers, nmax = max over neg layers
            def scatter_max(l0, nlayers, name):
                tiles = []
                for l in range(nlayers):
                    pt = psum.tile([NLOC, B], f32, tag=f"{name}{l}")
                    nc.tensor.matmul(pt[:], sc[:, l0 + l, :], bm[:],
                                     start=True, stop=True)
                    tiles.append(pt)
                # tensor_tensor may read at most one PSUM operand; do the
                # PSUM->SBUF copy on the otherwise-idle Scalar engine
                acc = pool.tile([NLOC, B], f32, tag=f"{name}acc")
                nc.scalar.copy(acc[:], tiles[0][:])
                for l in range(1, nlayers):
                    nxt = pool.tile([NLOC, B], f32, tag=f"{name}acc{l}")
                    nc.vector.tensor_tensor(
                        nxt[:], acc[:], tiles[l][:], mybir.AluOpType.max)
                    acc = nxt
                return acc

            lb = scatter_max(0, lpos, "sp")
            nmax = scatter_max(lpos, lneg, "sn")
            ub = pool.tile([NLOC, B], f32)
            nc.vector.tensor_scalar(
                ub[:], nmax[:], -1.0, 1.0, mybir.AluOpType.mult, mybir.AluOpType.add)

            lo = pool.tile([NLOC, B], f32)
            nc.vector.tensor_tensor(lo[:], lb[:], ub[:], mybir.AluOpType.min)
            hi = pool.tile([NLOC, B], f32)
            nc.vector.tensor_tensor(hi[:], lb[:], ub[:], mybir.AluOpType.max)
            mid = pool.tile([NLOC, B], f32)
            nc.vector.tensor_tensor(mid[:], hi[:], bs[:], mybir.AluOpType.min)
            u = pool.tile([NLOC, B], f32)
            nc.vector.tensor_tensor(u[:], lo[:], mid[:], mybir.AluOpType.max)
            nc.sync.dma_start(u_d[:], u[:])

    if SPLIT_WAITS:
        split_multi_waits(nc)
    _PROGRAM_CACHE[key] = nc
    return nc


class _Prep:
    """Host-side, input-value-independent-of-u preprocessing (everything that
    doesn't depend on intermediate u1)."""

    def __init__(self, preds, goal, atoms, pos_body, neg_body, pos_head, neg_head):
        f32 = np.float32
        self.atoms = np.asarray(atoms)
        self.p = preds[:, self.atoms].astype(f32)            # [B, NA]
        self.g = goal[:, self.atoms].astype(f32)
        self.pT = np.ascontiguousarray(self.p.T)             # [NA, B]
        self.gT = np.ascontiguousarray(self.g.T)

        import ml_dtypes
        self.bf16 = ml_dtypes.bfloat16
        sgT = (2.0 * self.g - 1.0).T                         # [NA, B]
        self.sg_dev = np.ascontiguousarray(
            sgT.reshape(KC, 128, B).transpose(1, 0, 2).reshape(128, KC * B)
        ).astype(self.bf16)

        hsum = pos_head + neg_head
        assert np.all(hsum.sum(axis=1) == 1.0), "heads must be one-hot"
        self.h = np.argmax(hsum, axis=1)                     # [C]
        self.head_is_pos = pos_head[np.arange(C), self.h] == 1.0
        owner = self.h // NLOC

        symm_body = (pos_body - neg_body).astype(f32)        # [C, NA]
        symm_head = (pos_head - neg_head).astype(f32)
        lit_count = (pos_body + neg_body).sum(axis=1).astype(f32)

        # literal row lists (row space: a -> v+ region, NA+a -> v- region)
        pos_lists = [np.nonzero(pos_body[c])[0] for c in range(C)]
        neg_lists = [np.nonzero(neg_body[c])[0] for c in range(C)]
        ncnt = np.array([len(pos_lists[c]) + len(neg_lists[c]) for c in range(C)])

        self.cons = []        # per core: constraint ids in slot order
        self.rows = []        # per core: [SLOTS, kpad] int row ids (-1 = pad)
        self.lhsTb = []       # per core: [128, KC*SLOTS] bf16 (symm_body)
        self.lhsTh = []       # per core: [128, KC*SLOTS] bf16 (symm_head)
        self.targ1 = []
        self.targ2 = []
        lpos_need, lneg_need = 1, 1
        layer_asn = []        # per core: (slot, is_pos, layer, nloc) list
        k_hi = k_lo = 1
        for i in range(NCORES):
            ci = np.nonzero(owner == i)[0]
            assert len(ci) <= SLOTS, f"core {i} has {len(ci)} constraints"
            # sort slots by literal count (desc): slots 64:128 then need far
            # fewer padded literal rows than slots 0:64
            ci = ci[np.argsort(-ncnt[ci], kind="stable")]
            self.cons.append(ci)
            cnts = ncnt[ci]
            k_hi = max(k_hi, int(cnts[:64].max(initial=0)))
            k_lo = max(k_lo, int(cnts[64:].max(initial=0)))
        self.kpad = (k_hi, k_lo)
        for i in range(NCORES):
            ci = self.cons[i]
            rows = -np.ones((SLOTS, k_hi), dtype=np.int64)
            for s, c in enumerate(ci):
                rr = np.concatenate([pos_lists[c], NA + neg_lists[c]])
                rows[s, : len(rr)] = rr
            self.rows.append(rows)

            def pack_lhsT(m):
                sl = np.zeros((NA, SLOTS), dtype=f32)
                sl[:, : len(ci)] = m[ci].T
                return np.ascontiguousarray(
                    sl.reshape(KC, 128, SLOTS).transpose(1, 0, 2)
                    .reshape(128, KC * SLOTS)).astype(self.bf16)

            self.lhsTb.append(pack_lhsT(symm_body))
            self.lhsTh.append(pack_lhsT(symm_head))
            t1 = np.full((SLOTS, 1), 1e9, dtype=f32)
            t1[: len(ci), 0] = lit_count[ci]
            self.targ1.append(t1)
            t2 = np.full((SLOTS, 1), 1e9, dtype=f32)
            t2[: len(ci), 0] = -1.0
            self.targ2.append(t2)

            # collision layers for the head scatter
            counts = {}
            asn = []
            for s, c in enumerate(ci):
                key = (self.h[c] % NLOC, bool(self.head_is_pos[c]))
                l = counts.get(key, 0)
                counts[key] = l + 1
                asn.append((s, key[1], l, key[0]))
                if key[1]:
                    lpos_need = max(lpos_need, l + 1)
                else:
                    lneg_need = max(lneg_need, l + 1)
            layer_asn.append(asn)

        self.lpos, self.lneg = lpos_need, lneg_need
        self.scat = []
        for i in range(NCORES):
            sc = np.zeros((SLOTS, self.lpos + self.lneg, NLOC), dtype=f32)
            for s, is_pos, l, n in layer_asn[i]:
                li = l if is_pos else self.lpos + l
                sc[s, li, n] = 1.0
            self.scat.append(np.ascontiguousarray(
                sc.reshape(SLOTS, -1)).astype(self.bf16))

    def vperm_maps(self, vcat: np.ndarray):
        """vcat: [2*NA, B] value table -> per-core (vph, vpl) f32 arrays."""
        k_hi, k_lo = self.kpad
        out = []
        vext = np.concatenate([vcat, np.zeros((1, B), np.float32)], axis=0)
        for i in range(NCORES):
            rows = self.rows[i]  # -1 pads -> last (zero) row
            g = vext[rows]       # [SLOTS, k_hi, B]
            vph = np.ascontiguousarray(
                g[:64].reshape(64, k_hi * B)).astype(np.float32)
            vpl = np.ascontiguousarray(
                g[64:, :k_lo].reshape(64, k_lo * B)).astype(np.float32)
            out.append((vph, vpl))
        return out


def kernel(preds, goal, atoms, pos_body, neg_body, pos_head, neg_head):
    preds = np.asarray(preds)
    prep = _Prep(np.asarray(preds, np.float32), np.asarray(goal, np.float32),
                 atoms, np.asarray(pos_body, np.float32),
                 np.asarray(neg_body, np.float32),
                 np.asarray(pos_head, np.float32),
                 np.asarray(neg_head, np.float32))
    nc = _build_program(prep.kpad, prep.lpos, prep.lneg)
    core_ids = list(range(NCORES))

    def launch(vcat, lhsT_list, targ_list, baseT):
        vperms = prep.vperm_maps(vcat)
        in_maps = []
        for i in range(NCORES):
            vph_i, vpl_i = vperms[i]
            in_maps.append({
                "vph": vph_i,
                "vpl": vpl_i,
                "sg": prep.sg_dev,
                "lhsT": lhsT_list[i],
                "targ": targ_list[i],
                "scat": prep.scat[i],
                "base": np.ascontiguousarray(
                    baseT[i * NLOC:(i + 1) * NLOC]).astype(np.float32),
            })
        res = run_bass_kernel_spmd(nc, in_maps, core_ids)
        return np.concatenate(
            [res.results[i]["u"] for i in range(NCORES)], axis=0)  # [NA, B]

    # apply 1: v+ = 1-p, v- = p, active vs lit_count, base = p
    vcat1 = np.concatenate([1.0 - prep.pT, prep.pT], axis=0)
    u1T = launch(vcat1, prep.lhsTb, prep.targ1, prep.pT)

    # apply 2: v+ = (1-g)(1-u1), v- = g*u1, active vs -1 (head), base = u1
    vcat2 = np.concatenate(
        [(1.0 - prep.gT) * (1.0 - u1T), prep.gT * u1T], axis=0
    ).astype(np.float32)
    u2T = launch(vcat2, prep.lhsTh, prep.targ2, u1T)

    out = np.array(preds, dtype=preds.dtype, copy=True)
    out[:, prep.atoms] = u2T.T.astype(preds.dtype)
    return out



# revision 11
# speedup vs baseline: 1.2103x; 1.2103x over previous
"""Trainium2 Bass kernel for nn_ConstraintsModule (fuzzy-logic constraint
propagation).

Algorithm notes
---------------
The reference computes, twice (apply-1 with active=full_body, apply-2 with
active=unsat_head and goal-masked bodies):

    body_rev[b,c,a] = pb[c,a] + v[b,a]*(nb-pb)      -> max over a
    body_min[b,c]   = active[b,c] * (1 - max_a body_rev)
    lb[b,n] = max_c body_min * pos_head[c,n] ; ub = 1 - max_c body_min*neg_head
    u = max(min(lb,ub), min(max(lb,ub), v))

Because bodies are sparse (~4 literals/constraint), heads one-hot, and
``active`` is 0/1 while ``1-W`` is in [0,1], the whole pre-scatter pipeline
collapses into ONE min-reduce over host-gathered literal rows:

    bm[c,b] = active*(1-W) = min(active, 1-W)
            = min over literal slots k of r[c,k,b]

with per-literal row values (pad rows = 1):
    apply-1  pos lit a: min(p[a], g[a])      neg lit a: min(1-p[a], 1-g[a])
    apply-2  pos lit a: max(g[a], u1[a])     neg lit a: 1 - min(g[a], u1[a])
    apply-2  head row : pos head: 1-g[h]     neg head: g[h]
(the act/unsat_head 0/1 factors are exactly the g-masks folded in above).

So the device program per launch is just:
    min-reduce [128 slots, K, B] -> bm        (DVE, bf16)
    one-hot head-scatter matmuls per collision layer (PE, bf16)
    max across layers, ub = 1-maxN, u = clip(base, min(lb,ub), max(lb,ub))
    (Activation/Pool/DVE share the elementwise tail)

Sharding: constraints are owned by the core that owns their head atom
(atom range of 128 per core), so the head-scatter and clamp are core-local.
ONE compiled SPMD program runs twice; the host rebuilds the gathered literal
rows from u1 between launches (pure data layout).
"""
import numpy as np

import concourse.bass as bass
import concourse.tile as tile
from concourse import mybir
from concourse.tile import ScopedClock
from concourse.bass_utils import run_bass_kernel_spmd

B = 128
NCOL = 2048
NA = 1024
C = 512
NCORES = 8
SLOTS = 128          # constraint slots per core (padded)
NLOC = 128           # atoms per core


class FixedTileContext(tile.TileContext):
    """Two workarounds for this walrus/NRT combo: (1) skip the tail
    clear_and_free_semaphores — its InstSemClear makes NRT reject the NEFF at
    load, and NRT resets semaphores per execution anyway; (2) multi-wait
    instructions are split afterwards by split_multi_waits()."""

    def _drain_and_barrier(self, tick_clock, wait_clock):
        drain_inst = self.nc.sync.drain()
        wait_clock.add_sem_waits(
            drain_inst.ins, ScopedClock({None: tick_clock.global_clock})
        )
        self.nc.all_engine_barrier()
        assert self.sems is not None
        popped = self.nc._tile_sem_poison_stack.pop()
        assert popped is self._sem_poison
        self.nc.all_engine_barrier()


def split_multi_waits(nc: bass.Bass) -> int:
    """walrus here accepts only ONE sync wait per instruction; Tile's
    add_semaphores attaches several.  Hoist all but one wait onto fresh
    same-engine nops placed immediately before the instruction (engine
    program order is preserved, so blocking semantics are identical)."""
    n_split = 0
    for f in nc.m.functions:
        for b in f.blocks:
            new = []
            for ins in b.instructions:
                si = ins.sync_info
                waits = list(si.on_wait) if si and si.on_wait else []
                if len(waits) > 1:
                    for w in waits[:-1]:
                        nop = mybir.InstNoOp(
                            name=f"waitsplit-{n_split}", ins=[], outs=[])
                        n_split += 1
                        nop.engine = ins.engine
                        nop.sync_info = mybir.SyncInfo(on_wait=[w], on_update=[])
                        new.append(nop)
                    ins.sync_info = mybir.SyncInfo(
                        on_wait=[waits[-1]],
                        on_update=list(si.on_update) if si.on_update else [])
                new.append(ins)
            b.instructions = new
    return n_split


_PROGRAM_CACHE = {}
SPLIT_WAITS = True  # set False when running under CoreSim (sim chokes on the
                    # synthesized nops, and doesn't need the split anyway)


def _build_program(kpad: tuple, lpos: int, lneg: int) -> bass.Bass:
    """One SPMD apply phase.  Inputs are per-core; the same program serves
    both applies (rg / scat contents differ per launch).

    Slot groups (slots sorted by literal count desc), each packed to use all
    128 partitions by splitting the batch:
      G0: slots  0:32,  K0 rows, partition = 32*q + s (q = b quarter)
      G1: slots 32:64,  K1 rows, same quartering
      G2: slots 64:128, K2 rows, partition = 64*q + s (q = b half)
    Rows are f32: tiny outputs arise as 1-(values near 1), so rows need
    ~1e-5 ABSOLUTE precision near 1.0 — no 16-bit float has that."""
    key = (kpad, lpos, lneg)
    if key in _PROGRAM_CACHE:
        return _PROGRAM_CACHE[key]

    f32, bf16 = mybir.dt.float32, mybir.dt.bfloat16
    K0, K1, K2 = kpad
    nc = bass.Bass(num_devices=NCORES)
    L = lpos + lneg
    o1 = K0 * 32
    o2 = o1 + K1 * 32
    ob = o2 + K2 * 64
    cols = ob + B
    rg_d = nc.declare_dram_parameter("rg", [128, cols], f32, isOutput=False)
    scat_d = nc.declare_dram_parameter("scat", [SLOTS, L * NLOC], bf16, isOutput=False)
    u_d = nc.declare_dram_parameter("u", [NLOC, B], f32, isOutput=True)

    with FixedTileContext(nc) as tc:
        with (
            tc.tile_pool(name="sbuf", bufs=1) as pool,
            tc.tile_pool(name="psum", bufs=1, space="PSUM") as psum,
        ):
            rg = pool.tile([128, cols], f32)
            nc.sync.dma_start(rg[:], rg_d[:])
            sc = pool.tile([SLOTS, L, NLOC], bf16)
            nc.scalar.dma_start(
                sc[:], scat_d[:].rearrange("p (l n) -> p l n", l=L))

            # PE pstate warmup: a tiny matmul on zeroed data during the DMA
            # wait lifts the real matmuls from the cold 0.65GHz pstate
            wz = pool.tile([128, 1], bf16)
            nc.gpsimd.memset(wz[:], 0.0)
            wp = psum.tile([1, 1], f32, tag="warm")
            nc.tensor.matmul(wp[:], wz[:], wz[:], start=True, stop=True)

            base = rg[:, ob:]  # [NLOC, B] f32, local p / u1

            # w[s,b] = min over literal rows per group (pad rows are 1.0)
            w0 = pool.tile([128, 32], f32)
            nc.vector.tensor_reduce(
                out=w0[:], in_=rg[:, :o1].rearrange("p (k b) -> p b k", k=K0),
                axis=mybir.AxisListType.X, op=mybir.AluOpType.min)
            w1 = pool.tile([128, 32], f32)
            nc.vector.tensor_reduce(
                out=w1[:], in_=rg[:, o1:o2].rearrange("p (k b) -> p b k", k=K1),
                axis=mybir.AxisListType.X, op=mybir.AluOpType.min)
            w2 = pool.tile([128, 64], f32)
            nc.vector.tensor_reduce(
                out=w2[:], in_=rg[:, o2:ob].rearrange("p (k b) -> p b k", k=K2),
                axis=mybir.AxisListType.X, op=mybir.AluOpType.min)

            # reassemble bm[slot, b] with partition-shifted copies (Act)
            bm = pool.tile([SLOTS, B], f32)
            for q in range(4):
                nc.scalar.copy(bm[0:32, 32 * q:32 * q + 32],
                               w0[32 * q:32 * q + 32, :])
                nc.scalar.copy(bm[32:64, 32 * q:32 * q + 32],
                               w1[32 * q:32 * q + 32, :])
            for q in range(2):
                nc.scalar.copy(bm[64:128, 64 * q:64 * q + 64],
                               w2[64 * q:64 * q + 64, :])

            # exact bf16 hi/lo split of bm: the one-hot scatter matmuls run
            # at bf16 speed while psum accumulates the f32-accurate sum
            bmh = pool.tile([SLOTS, B], bf16)
            nc.scalar.copy(bmh[:], bm[:])
            nbmh = pool.tile([SLOTS, B], bf16)
            nc.scalar.activation(
                nbmh[:], bm[:], mybir.ActivationFunctionType.Copy, scale=-1.0)
            bml = pool.tile([SLOTS, B], bf16)
            nc.vector.tensor_tensor(bml[:], bm[:], nbmh[:], mybir.AluOpType.add)

            # head scatter: per collision layer, 2 accumulating bf16 matmuls;
            # each sign's layers share a PSUM bank so the cross-layer max is
            # ONE tensor_reduce
            def layer_group(l0, n, name):
                pt = psum.tile([NLOC, n * B], f32, tag=name)
                for l in range(n):
                    nc.tensor.matmul(pt[:, l * B:(l + 1) * B],
                                     sc[:, l0 + l, :], bmh[:],
                                     start=True, stop=False)
                    nc.tensor.matmul(pt[:, l * B:(l + 1) * B],
                                     sc[:, l0 + l, :], bml[:],
                                     start=False, stop=True)
                acc = pool.tile([NLOC, B], f32, tag=f"{name}acc")
                if n == 1:
                    nc.scalar.copy(acc[:], pt[:])
                else:
                    nc.vector.tensor_reduce(
                        out=acc[:],
                        in_=pt[:].rearrange("p (l b) -> p b l", l=n),
                        axis=mybir.AxisListType.X, op=mybir.AluOpType.max)
                return acc

            lb = layer_group(0, lpos, "sp")
            nmax = layer_group(lpos, lneg, "sn")
            # ub = 1 - nmax on the Activation engine: Copy(in*-1 + 1)
            ub = pool.tile([NLOC, B], f32)
            nc.scalar.activation(
                ub[:], nmax[:], mybir.ActivationFunctionType.Copy,
                bias=1.0, scale=-1.0)

            # u = clip(base, min(lb,ub), max(lb,ub))
            lo = pool.tile([NLOC, B], f32)
            nc.vector.tensor_tensor(lo[:], lb[:], ub[:], mybir.AluOpType.min)
            hi = pool.tile([NLOC, B], f32)
            nc.vector.tensor_tensor(hi[:], lb[:], ub[:], mybir.AluOpType.max)
            mid = pool.tile([NLOC, B], f32)
            nc.vector.tensor_tensor(mid[:], hi[:], base[:], mybir.AluOpType.min)
            u = pool.tile([NLOC, B], f32)
            nc.vector.tensor_tensor(u[:], lo[:], mid[:], mybir.AluOpType.max)
            nc.sync.dma_start(u_d[:], u[:])

    if SPLIT_WAITS:
        split_multi_waits(nc)
    _PROGRAM_CACHE[key] = nc
    return nc


class _Prep:
    """Host-side preprocessing: slot assignment, literal row-index tables,
    one-hot scatter layers.  Everything independent of u1."""

    def __init__(self, preds, goal, atoms, pos_body, neg_body, pos_head, neg_head):
        f32 = np.float32
        import ml_dtypes
        self.bf16 = ml_dtypes.bfloat16
        self.atoms = np.asarray(atoms)
        self.p = preds[:, self.atoms].astype(f32)            # [B, NA]
        self.g = goal[:, self.atoms].astype(f32)
        self.pT = np.ascontiguousarray(self.p.T)             # [NA, B]
        self.gT = np.ascontiguousarray(self.g.T)

        hsum = pos_head + neg_head
        assert np.all(hsum.sum(axis=1) == 1.0), "heads must be one-hot"
        self.h = np.argmax(hsum, axis=1)                     # [C]
        head_is_pos = pos_head[np.arange(C), self.h] == 1.0
        owner = self.h // NLOC

        pos_lists = [np.nonzero(pos_body[c])[0] for c in range(C)]
        neg_lists = [np.nonzero(neg_body[c])[0] for c in range(C)]
        ncnt = np.array([len(pos_lists[c]) + len(neg_lists[c]) for c in range(C)])

        PAD = 4 * NA  # index of the constant-1.0 row in vext

        self.rows1 = []       # per core: [SLOTS, Kmax] row ids for apply-1
        self.rows2 = []       # per core: [SLOTS, Kmax] row ids for apply-2
        lpos_need, lneg_need = 1, 1
        layer_asn = []
        slot_cnt = np.zeros((NCORES, SLOTS), dtype=int)
        Kmax = int(ncnt.max()) + 1
        for i in range(NCORES):
            ci = np.nonzero(owner == i)[0]
            assert len(ci) <= SLOTS, f"core {i} has {len(ci)} constraints"
            ci = ci[np.argsort(-ncnt[ci], kind="stable")]
            slot_cnt[i, : len(ci)] = ncnt[ci] + 1  # +1 head row
            r1 = np.full((SLOTS, Kmax), PAD, dtype=np.int64)
            r2 = np.full((SLOTS, Kmax), PAD, dtype=np.int64)
            for s, c in enumerate(ci):
                rr = np.concatenate([pos_lists[c], NA + neg_lists[c]])
                r1[s, : len(rr)] = rr
                r2[s, : len(rr)] = rr
                hrow = (2 * NA if head_is_pos[c] else 3 * NA) + self.h[c]
                r2[s, len(rr)] = hrow
            self.rows1.append(r1)
            self.rows2.append(r2)

            counts = {}
            asn = []
            for s, c in enumerate(ci):
                key = (self.h[c] % NLOC, bool(head_is_pos[c]))
                l = counts.get(key, 0)
                counts[key] = l + 1
                asn.append((s, key[1], l, key[0]))
                if key[1]:
                    lpos_need = max(lpos_need, l + 1)
                else:
                    lneg_need = max(lneg_need, l + 1)
            layer_asn.append(asn)

        self.lpos, self.lneg = lpos_need, lneg_need
        # per-group row pads (slots sorted desc => group max is at its start)
        self.kpad = (int(slot_cnt[:, 0:32].max()),
                     int(slot_cnt[:, 32:64].max(initial=1) or 1),
                     int(slot_cnt[:, 64:128].max(initial=1) or 1))
        self.kpad = tuple(max(k, 1) for k in self.kpad)
        self.scat = []
        for i in range(NCORES):
            sc = np.zeros((SLOTS, self.lpos + self.lneg, NLOC), dtype=f32)
            for s, is_pos, l, n in layer_asn[i]:
                li = l if is_pos else self.lpos + l
                sc[s, li, n] = 1.0
            self.scat.append(np.ascontiguousarray(
                sc.reshape(SLOTS, -1)).astype(self.bf16))

    def rg_maps(self, vext: np.ndarray, rows_list, baseT: np.ndarray):
        """vext: [4*NA+1, B] value table -> per-core rg arrays [128, cols]."""
        K0, K1, K2 = self.kpad
        cols = K0 * 32 + K1 * 32 + K2 * 64 + B
        out = []
        for i in range(NCORES):
            gat = vext[rows_list[i]]                     # [SLOTS, Kmax, B]
            rg = np.empty((128, cols), dtype=np.float32)
            # G0: partition 32q+s <- slot s rows, batch 32q:32q+32
            g0 = gat[0:32, :K0, :].reshape(32, K0, 4, 32)
            rg[:, : K0 * 32] = np.ascontiguousarray(
                g0.transpose(2, 0, 1, 3)).reshape(128, K0 * 32)
            o = K0 * 32
            g1 = gat[32:64, :K1, :].reshape(32, K1, 4, 32)
            rg[:, o:o + K1 * 32] = np.ascontiguousarray(
                g1.transpose(2, 0, 1, 3)).reshape(128, K1 * 32)
            o += K1 * 32
            g2 = gat[64:128, :K2, :].reshape(64, K2, 2, 64)
            rg[:, o:o + K2 * 64] = np.ascontiguousarray(
                g2.transpose(2, 0, 1, 3)).reshape(128, K2 * 64)
            o += K2 * 64
            rg[:, o:] = baseT[i * NLOC:(i + 1) * NLOC]
            out.append(rg)
        return out


def kernel(preds, goal, atoms, pos_body, neg_body, pos_head, neg_head):
    preds = np.asarray(preds)
    prep = _Prep(np.asarray(preds, np.float32), np.asarray(goal, np.float32),
                 atoms, np.asarray(pos_body, np.float32),
                 np.asarray(neg_body, np.float32),
                 np.asarray(pos_head, np.float32),
                 np.asarray(neg_head, np.float32))
    nc = _build_program(prep.kpad, prep.lpos, prep.lneg)
    core_ids = list(range(NCORES))

    def launch(vext, rows_list, baseT):
        rgs = prep.rg_maps(vext, rows_list, baseT)
        in_maps = [{"rg": rgs[i], "scat": prep.scat[i]} for i in range(NCORES)]
        res = run_bass_kernel_spmd(nc, in_maps, core_ids)
        return np.concatenate(
            [res.results[i]["u"] for i in range(NCORES)], axis=0)  # [NA, B]

    f32 = np.float32
    ones = np.ones((1, B), f32)
    pT, gT = prep.pT, prep.gT

    # apply 1: pos lit -> min(p,g); neg lit -> min(1-p,1-g); heads unused
    vext1 = np.concatenate(
        [np.minimum(pT, gT), np.minimum(1.0 - pT, 1.0 - gT),
         np.ones((2 * NA, B), f32), ones], axis=0)
    u1T = launch(vext1, prep.rows1, pT)

    # apply 2: pos lit -> max(g,u1); neg lit -> 1-min(g,u1);
    #          head row -> pos: 1-g, neg: g
    vext2 = np.concatenate(
        [np.maximum(gT, u1T), 1.0 - np.minimum(gT, u1T),
         1.0 - gT, gT, ones], axis=0).astype(f32)
    u2T = launch(vext2, prep.rows2, u1T)

    out = np.array(preds, dtype=preds.dtype, copy=True)
    out[:, prep.atoms] = u2T.T.astype(preds.dtype)
    return out


# revision 15
# speedup vs baseline: 1.3563x; 1.1206x over previous
"""Trainium2 Bass kernel for nn_ConstraintsModule (fuzzy-logic constraint
propagation).

Algorithm notes
---------------
The reference computes, twice (apply-1 with active=full_body, apply-2 with
active=unsat_head and goal-masked bodies):

    body_rev[b,c,a] = pb[c,a] + v[b,a]*(nb-pb)      -> max over a
    body_min[b,c]   = active[b,c] * (1 - max_a body_rev)
    lb[b,n] = max_c body_min * pos_head[c,n] ; ub = 1 - max_c body_min*neg_head
    u = max(min(lb,ub), min(max(lb,ub), v))

Because bodies are sparse (~4 literals/constraint), heads one-hot, and
``active`` is 0/1 while ``1-W`` is in [0,1], the whole pre-scatter pipeline
collapses into ONE min-reduce over host-gathered literal rows:

    bm[c,b] = active*(1-W) = min(active, 1-W)
            = min over literal slots k of r[c,k,b]

with per-literal row values (pad rows = 1):
    apply-1  pos lit a: min(p[a], g[a])      neg lit a: min(1-p[a], 1-g[a])
    apply-2  pos lit a: max(g[a], u1[a])     neg lit a: 1 - min(g[a], u1[a])
    apply-2  head row : pos head: 1-g[h]     neg head: g[h]
(the act/unsat_head 0/1 factors are exactly the g-masks folded in above).

So the device program per launch is just:
    min-reduce [128 slots, K, B] -> bm        (DVE, bf16)
    one-hot head-scatter matmuls per collision layer (PE, bf16)
    max across layers, ub = 1-maxN, u = clip(base, min(lb,ub), max(lb,ub))
    (Activation/Pool/DVE share the elementwise tail)

Sharding: constraints are owned by the core that owns their head atom
(atom range of 128 per core), so the head-scatter and clamp are core-local.
ONE compiled SPMD program runs twice; the host rebuilds the gathered literal
rows from u1 between launches (pure data layout).
"""
import numpy as np

import concourse.bass as bass
import concourse.tile as tile
from concourse import mybir
from concourse.tile import ScopedClock
from concourse.bass_utils import run_bass_kernel_spmd

B = 128
NCOL = 2048
NA = 1024
C = 512
NCORES = 8
SLOTS = 128          # constraint slots per core (padded)
NLOC = 128           # atoms per core


class FixedTileContext(tile.TileContext):
    """Two workarounds for this walrus/NRT combo: (1) skip the tail
    clear_and_free_semaphores — its InstSemClear makes NRT reject the NEFF at
    load, and NRT resets semaphores per execution anyway; (2) multi-wait
    instructions are split afterwards by split_multi_waits()."""

    def _drain_and_barrier(self, tick_clock, wait_clock):
        drain_inst = self.nc.sync.drain()
        wait_clock.add_sem_waits(
            drain_inst.ins, ScopedClock({None: tick_clock.global_clock})
        )
        self.nc.all_engine_barrier()
        assert self.sems is not None
        popped = self.nc._tile_sem_poison_stack.pop()
        assert popped is self._sem_poison
        self.nc.all_engine_barrier()


def split_multi_waits(nc: bass.Bass) -> int:
    """walrus here accepts only ONE sync wait per instruction; Tile's
    add_semaphores attaches several.  Hoist all but one wait onto fresh
    same-engine nops placed immediately before the instruction (engine
    program order is preserved, so blocking semantics are identical)."""
    n_split = 0
    for f in nc.m.functions:
        for b in f.blocks:
            new = []
            for ins in b.instructions:
                si = ins.sync_info
                waits = list(si.on_wait) if si and si.on_wait else []
                if len(waits) > 1:
                    for w in waits[:-1]:
                        nop = mybir.InstNoOp(
                            name=f"waitsplit-{n_split}", ins=[], outs=[])
                        n_split += 1
                        nop.engine = ins.engine
                        nop.sync_info = mybir.SyncInfo(on_wait=[w], on_update=[])
                        new.append(nop)
                    ins.sync_info = mybir.SyncInfo(
                        on_wait=[waits[-1]],
                        on_update=list(si.on_update) if si.on_update else [])
                new.append(ins)
            b.instructions = new
    return n_split


_PROGRAM_CACHE = {}
SPLIT_WAITS = True  # set False when running under CoreSim (sim chokes on the
                    # synthesized nops, and doesn't need the split anyway)


def _build_program(kpad: tuple, lpos: int, lneg: int) -> bass.Bass:
    """One SPMD apply phase.  Inputs are per-core; the same program serves
    both applies (rg / scat contents differ per launch).

    Slot groups (slots sorted by literal count desc), each packed to use all
    128 partitions by splitting the batch:
      G0: slots  0:32,  K0 rows, partition = 32*q + s (q = b quarter)
      G1: slots 32:64,  K1 rows, same quartering
      G2: slots 64:128, K2 rows, partition = 64*q + s (q = b half)
    Rows are f32: tiny outputs arise as 1-(values near 1), so rows need
    ~1e-5 ABSOLUTE precision near 1.0 — no 16-bit float has that."""
    key = (kpad, lpos, lneg)
    if key in _PROGRAM_CACHE:
        return _PROGRAM_CACHE[key]

    f32, bf16 = mybir.dt.float32, mybir.dt.bfloat16
    K0, K1, K2 = kpad
    nc = bass.Bass(num_devices=NCORES)
    L = lpos + lneg
    s0, s1, s2 = K0 * 32, K1 * 32, K2 * 64
    rg0_d = nc.declare_dram_parameter("rg0", [128, s0], f32, isOutput=False)
    rg1_d = nc.declare_dram_parameter("rg1", [128, s1], f32, isOutput=False)
    rg2_d = nc.declare_dram_parameter("rg2", [128, s2], f32, isOutput=False)
    scat_d = nc.declare_dram_parameter("scat", [SLOTS, L * NLOC], bf16, isOutput=False)
    base_d = nc.declare_dram_parameter("base", [NLOC, B], f32, isOutput=False)
    u_d = nc.declare_dram_parameter("u", [NLOC, B], f32, isOutput=True)

    with FixedTileContext(nc) as tc:
        with (
            tc.tile_pool(name="sbuf", bufs=1) as pool,
            tc.tile_pool(name="psum", bufs=1, space="PSUM") as psum,
        ):
            # 3 sync-ring DMAs so group-0's reduce starts as soon as its own
            # chunk lands; scat/base ride the scalar ring in parallel
            rg0 = pool.tile([128, s0], f32)
            nc.sync.dma_start(rg0[:], rg0_d[:])
            rg1 = pool.tile([128, s1], f32)
            nc.sync.dma_start(rg1[:], rg1_d[:])
            rg2 = pool.tile([128, s2], f32)
            nc.sync.dma_start(rg2[:], rg2_d[:])
            sc = pool.tile([SLOTS, L, NLOC], bf16)
            nc.scalar.dma_start(
                sc[:], scat_d[:].rearrange("p (l n) -> p l n", l=L))
            base = pool.tile([NLOC, B], f32)
            nc.scalar.dma_start(base[:], base_d[:])

            # PE pstate warmup: a tiny matmul on zeroed data during the DMA
            # wait lifts the real matmuls from the cold 0.65GHz pstate
            wz = pool.tile([128, 1], bf16)
            nc.gpsimd.memset(wz[:], 0.0)
            wp = psum.tile([1, 1], f32, tag="warm")
            nc.tensor.matmul(wp[:], wz[:], wz[:], start=True, stop=True)

            # w[s,b] = min over literal rows per group (pad rows are 1.0)
            w0 = pool.tile([128, 32], f32)
            nc.vector.tensor_reduce(
                out=w0[:], in_=rg0[:].rearrange("p (k b) -> p b k", k=K0),
                axis=mybir.AxisListType.X, op=mybir.AluOpType.min)
            w1 = pool.tile([128, 32], f32)
            nc.vector.tensor_reduce(
                out=w1[:], in_=rg1[:].rearrange("p (k b) -> p b k", k=K1),
                axis=mybir.AxisListType.X, op=mybir.AluOpType.min)
            w2 = pool.tile([128, 64], f32)
            nc.vector.tensor_reduce(
                out=w2[:], in_=rg2[:].rearrange("p (k b) -> p b k", k=K2),
                axis=mybir.AxisListType.X, op=mybir.AluOpType.min)

            # reassemble bm[slot, b] with partition-shifted copies, split
            # between Act (group 0, available earliest) and DVE (rest)
            bm = pool.tile([SLOTS, B], f32)
            for q in range(4):
                nc.scalar.copy(bm[0:32, 32 * q:32 * q + 32],
                               w0[32 * q:32 * q + 32, :])
                nc.vector.tensor_scalar(
                    bm[32:64, 32 * q:32 * q + 32],
                    w1[32 * q:32 * q + 32, :],
                    0.0, None, mybir.AluOpType.add)
            for q in range(2):
                nc.vector.tensor_scalar(
                    bm[64:128, 64 * q:64 * q + 64],
                    w2[64 * q:64 * q + 64, :],
                    0.0, None, mybir.AluOpType.add)

            # exact bf16 hi/lo split of bm: the one-hot scatter matmuls run
            # at bf16 speed while psum accumulates the f32-accurate sum
            bmh = pool.tile([SLOTS, B], bf16)
            nc.vector.tensor_scalar(
                bmh[:], bm[:], 0.0, None, mybir.AluOpType.add)
            nbmh = pool.tile([SLOTS, B], bf16)
            nc.scalar.activation(
                nbmh[:], bm[:], mybir.ActivationFunctionType.Copy, scale=-1.0)
            bml = pool.tile([SLOTS, B], bf16)
            nc.vector.tensor_tensor(bml[:], bm[:], nbmh[:], mybir.AluOpType.add)

            # head scatter: per collision layer, 2 accumulating bf16 matmuls;
            # each sign's layers share a PSUM bank so the cross-layer max is
            # ONE tensor_reduce
            def layer_group(l0, n, name):
                pt = psum.tile([NLOC, n * B], f32, tag=name)
                for l in range(n):
                    nc.tensor.matmul(pt[:, l * B:(l + 1) * B],
                                     sc[:, l0 + l, :], bmh[:],
                                     start=True, stop=False)
                    nc.tensor.matmul(pt[:, l * B:(l + 1) * B],
                                     sc[:, l0 + l, :], bml[:],
                                     start=False, stop=True)
                acc = pool.tile([NLOC, B], f32, tag=f"{name}acc")
                if n == 1:
                    nc.scalar.copy(acc[:], pt[:])
                else:
                    nc.vector.tensor_reduce(
                        out=acc[:],
                        in_=pt[:].rearrange("p (l b) -> p b l", l=n),
                        axis=mybir.AxisListType.X, op=mybir.AluOpType.max)
                return acc

            lb = layer_group(0, lpos, "sp")
            nmax = layer_group(lpos, lneg, "sn")
            # ub = 1 - nmax on DVE (keeps the tail chain on one engine)
            ub = pool.tile([NLOC, B], f32)
            nc.vector.tensor_scalar(
                ub[:], nmax[:], -1.0, 1.0,
                mybir.AluOpType.mult, mybir.AluOpType.add)

            # u = clip(base, min(lb,ub), max(lb,ub))
            lo = pool.tile([NLOC, B], f32)
            nc.vector.tensor_tensor(lo[:], lb[:], ub[:], mybir.AluOpType.min)
            hi = pool.tile([NLOC, B], f32)
            nc.vector.tensor_tensor(hi[:], lb[:], ub[:], mybir.AluOpType.max)
            mid = pool.tile([NLOC, B], f32)
            nc.vector.tensor_tensor(mid[:], hi[:], base[:], mybir.AluOpType.min)
            u = pool.tile([NLOC, B], f32)
            nc.vector.tensor_tensor(u[:], lo[:], mid[:], mybir.AluOpType.max)
            nc.sync.dma_start(u_d[:], u[:])

    if SPLIT_WAITS:
        split_multi_waits(nc)
    _PROGRAM_CACHE[key] = nc
    return nc


class _Prep:
    """Host-side preprocessing: slot assignment, literal row-index tables,
    one-hot scatter layers.  Everything independent of u1."""

    def __init__(self, preds, goal, atoms, pos_body, neg_body, pos_head, neg_head):
        f32 = np.float32
        import ml_dtypes
        self.bf16 = ml_dtypes.bfloat16
        self.atoms = np.asarray(atoms)
        self.p = preds[:, self.atoms].astype(f32)            # [B, NA]
        self.g = goal[:, self.atoms].astype(f32)
        self.pT = np.ascontiguousarray(self.p.T)             # [NA, B]
        self.gT = np.ascontiguousarray(self.g.T)

        hsum = pos_head + neg_head
        assert np.all(hsum.sum(axis=1) == 1.0), "heads must be one-hot"
        self.h = np.argmax(hsum, axis=1)                     # [C]
        head_is_pos = pos_head[np.arange(C), self.h] == 1.0
        owner = self.h // NLOC

        pos_lists = [np.nonzero(pos_body[c])[0] for c in range(C)]
        neg_lists = [np.nonzero(neg_body[c])[0] for c in range(C)]
        ncnt = np.array([len(pos_lists[c]) + len(neg_lists[c]) for c in range(C)])

        PAD = 4 * NA  # index of the constant-1.0 row in vext

        self.rows1 = []       # per core: [SLOTS, Kmax] row ids for apply-1
        self.rows2 = []       # per core: [SLOTS, Kmax] row ids for apply-2
        lpos_need, lneg_need = 1, 1
        layer_asn = []
        slot_cnt = np.zeros((NCORES, SLOTS), dtype=int)
        Kmax = int(ncnt.max()) + 1
        for i in range(NCORES):
            ci = np.nonzero(owner == i)[0]
            assert len(ci) <= SLOTS, f"core {i} has {len(ci)} constraints"
            ci = ci[np.argsort(-ncnt[ci], kind="stable")]
            slot_cnt[i, : len(ci)] = ncnt[ci] + 1  # +1 head row
            r1 = np.full((SLOTS, Kmax), PAD, dtype=np.int64)
            r2 = np.full((SLOTS, Kmax), PAD, dtype=np.int64)
            for s, c in enumerate(ci):
                rr = np.concatenate([pos_lists[c], NA + neg_lists[c]])
                r1[s, : len(rr)] = rr
                r2[s, : len(rr)] = rr
                hrow = (2 * NA if head_is_pos[c] else 3 * NA) + self.h[c]
                r2[s, len(rr)] = hrow
            self.rows1.append(r1)
            self.rows2.append(r2)

            counts = {}
            asn = []
            for s, c in enumerate(ci):
                key = (self.h[c] % NLOC, bool(head_is_pos[c]))
                l = counts.get(key, 0)
                counts[key] = l + 1
                asn.append((s, key[1], l, key[0]))
                if key[1]:
                    lpos_need = max(lpos_need, l + 1)
                else:
                    lneg_need = max(lneg_need, l + 1)
            layer_asn.append(asn)

        self.lpos, self.lneg = lpos_need, lneg_need
        # per-group row pads (slots sorted desc => group max is at its start)
        self.kpad = (int(slot_cnt[:, 0:32].max()),
                     int(slot_cnt[:, 32:64].max(initial=1) or 1),
                     int(slot_cnt[:, 64:128].max(initial=1) or 1))
        self.kpad = tuple(max(k, 1) for k in self.kpad)
        self.scat = []
        for i in range(NCORES):
            sc = np.zeros((SLOTS, self.lpos + self.lneg, NLOC), dtype=f32)
            for s, is_pos, l, n in layer_asn[i]:
                li = l if is_pos else self.lpos + l
                sc[s, li, n] = 1.0
            self.scat.append(np.ascontiguousarray(
                sc.reshape(SLOTS, -1)).astype(self.bf16))

    def rg_maps(self, vext: np.ndarray, rows_list):
        """vext: [4*NA+1, B] value table -> per-core (rg0, rg1, rg2) arrays.
        Group packing: partition = b_chunk * group_width + local_slot."""
        K0, K1, K2 = self.kpad
        out = []
        for i in range(NCORES):
            gat = vext[rows_list[i]]                     # [SLOTS, Kmax, B]
            g0 = np.ascontiguousarray(
                gat[0:32, :K0, :].reshape(32, K0, 4, 32)
                .transpose(2, 0, 1, 3)).reshape(128, K0 * 32)
            g1 = np.ascontiguousarray(
                gat[32:64, :K1, :].reshape(32, K1, 4, 32)
                .transpose(2, 0, 1, 3)).reshape(128, K1 * 32)
            g2 = np.ascontiguousarray(
                gat[64:128, :K2, :].reshape(64, K2, 2, 64)
                .transpose(2, 0, 1, 3)).reshape(128, K2 * 64)
            out.append((g0, g1, g2))
        return out


def kernel(preds, goal, atoms, pos_body, neg_body, pos_head, neg_head):
    preds = np.asarray(preds)
    prep = _Prep(np.asarray(preds, np.float32), np.asarray(goal, np.float32),
                 atoms, np.asarray(pos_body, np.float32),
                 np.asarray(neg_body, np.float32),
                 np.asarray(pos_head, np.float32),
                 np.asarray(neg_head, np.float32))
    nc = _build_program(prep.kpad, prep.lpos, prep.lneg)
    core_ids = list(range(NCORES))

    def launch(vext, rows_list, baseT):
        rgs = prep.rg_maps(vext, rows_list)
        in_maps = [{"rg0": rgs[i][0], "rg1": rgs[i][1], "rg2": rgs[i][2],
                    "scat": prep.scat[i],
                    "base": np.ascontiguousarray(
                        baseT[i * NLOC:(i + 1) * NLOC]).astype(np.float32)}
                   for i in range(NCORES)]
        res = run_bass_kernel_spmd(nc, in_maps, core_ids)
        return np.concatenate(
            [res.results[i]["u"] for i in range(NCORES)], axis=0)  # [NA, B]

    f32 = np.float32
    ones = np.ones((1, B), f32)
    pT, gT = prep.pT, prep.gT

    # apply 1: pos lit -> min(p,g); neg lit -> min(1-p,1-g); heads unused
    vext1 = np.concatenate(
        [np.minimum(pT, gT), np.minimum(1.0 - pT, 1.0 - gT),
         np.ones((2 * NA, B), f32), ones], axis=0)
    u1T = launch(vext1, prep.rows1, pT)

    # apply 2: pos lit -> max(g,u1); neg lit -> 1-min(g,u1);
    #          head row -> pos: 1-g, neg: g
    vext2 = np.concatenate(
        [np.maximum(gT, u1T), 1.0 - np.minimum(gT, u1T),
         1.0 - gT, gT, ones], axis=0).astype(f32)
    u2T = launch(vext2, prep.rows2, u1T)

    out = np.array(preds, dtype=preds.dtype, copy=True)
    out[:, prep.atoms] = u2T.T.astype(preds.dtype)
    return out


# revision 33
# speedup vs baseline: 1.5346x; 1.1315x over previous
"""Trainium2 Bass kernel for nn_ConstraintsModule (fuzzy-logic constraint
propagation).

Algorithm notes
---------------
The reference computes, twice (apply-1 with active=full_body, apply-2 with
active=unsat_head and goal-masked bodies):

    body_rev[b,c,a] = pb[c,a] + v[b,a]*(nb-pb)      -> max over a
    body_min[b,c]   = active[b,c] * (1 - max_a body_rev)
    lb[b,n] = max_c body_min * pos_head[c,n] ; ub = 1 - max_c body_min*neg_head
    u = max(min(lb,ub), min(max(lb,ub), v))

Because bodies are sparse (~4 literals/constraint), heads one-hot, and
``active`` is 0/1 while ``1-W`` is in [0,1], the whole pre-scatter pipeline
collapses into ONE min-reduce over host-gathered literal rows:

    bm[c,b] = active*(1-W) = min(active, 1-W)
            = min over literal slots k of r[c,k,b]

with per-literal row values (pad rows = 1):
    apply-1  pos lit a: min(p[a], g[a])      neg lit a: min(1-p[a], 1-g[a])
    apply-2  pos lit a: max(g[a], u1[a])     neg lit a: 1 - min(g[a], u1[a])
    apply-2  head row : pos head: 1-g[h]     neg head: g[h]
(the act/unsat_head 0/1 factors are exactly the g-masks folded in above).

So the device program per launch is just:
    min-reduce [128 slots, K, B] -> bm        (DVE, bf16)
    one-hot head-scatter matmuls per collision layer (PE, bf16)
    max across layers, ub = 1-maxN, u = clip(base, min(lb,ub), max(lb,ub))
    (Activation/Pool/DVE share the elementwise tail)

Sharding: constraints are owned by the core that owns their head atom
(atom range of 128 per core), so the head-scatter and clamp are core-local.
ONE compiled SPMD program runs twice; the host rebuilds the gathered literal
rows from u1 between launches (pure data layout).
"""
import numpy as np

import concourse.bass as bass
import concourse.tile as tile
from concourse import mybir
from concourse.tile import ScopedClock
from concourse.bass_utils import run_bass_kernel_spmd

B = 128
NCOL = 2048
NA = 1024
C = 512
NCORES = 8
SLOTS = 128          # constraint slots per core (padded)
NLOC = 128           # atoms per core


class FixedTileContext(tile.TileContext):
    """Two workarounds for this walrus/NRT combo: (1) skip the tail
    clear_and_free_semaphores — its InstSemClear makes NRT reject the NEFF at
    load, and NRT resets semaphores per execution anyway; (2) multi-wait
    instructions are split afterwards by split_multi_waits()."""

    def _drain_and_barrier(self, tick_clock, wait_clock):
        drain_inst = self.nc.sync.drain()
        wait_clock.add_sem_waits(
            drain_inst.ins, ScopedClock({None: tick_clock.global_clock})
        )
        self.nc.all_engine_barrier()
        assert self.sems is not None
        popped = self.nc._tile_sem_poison_stack.pop()
        assert popped is self._sem_poison


def split_multi_waits(nc: bass.Bass) -> int:
    """walrus here accepts only ONE sync wait per instruction; Tile's
    add_semaphores attaches several.  Hoist all but one wait onto fresh
    same-engine nops placed immediately before the instruction (engine
    program order is preserved, so blocking semantics are identical)."""
    n_split = 0
    for f in nc.m.functions:
        for b in f.blocks:
            new = []
            for ins in b.instructions:
                si = ins.sync_info
                waits = list(si.on_wait) if si and si.on_wait else []
                if len(waits) > 1:
                    for w in waits[:-1]:
                        nop = mybir.InstNoOp(
                            name=f"waitsplit-{n_split}", ins=[], outs=[])
                        n_split += 1
                        nop.engine = ins.engine
                        nop.sync_info = mybir.SyncInfo(on_wait=[w], on_update=[])
                        new.append(nop)
                    ins.sync_info = mybir.SyncInfo(
                        on_wait=[waits[-1]],
                        on_update=list(si.on_update) if si.on_update else [])
                new.append(ins)
            b.instructions = new
    return n_split


_PROGRAM_CACHE = {}
SPLIT_WAITS = True  # set False when running under CoreSim (sim chokes on the
                    # synthesized nops, and doesn't need the split anyway)


def _build_program(kpad: tuple, lpos: int, lneg: int) -> bass.Bass:
    """One SPMD apply phase.  Inputs are per-core; the same program serves
    both applies (rg / scat contents differ per launch).

    Slot groups (slots sorted by literal count desc), each packed to use all
    128 partitions by splitting the batch:
      G0: slots  0:32,  K0 rows, partition = 32*q + s (q = b quarter)
      G1: slots 32:64,  K1 rows, same quartering
      G2: slots 64:128, K2 rows, partition = 64*q + s (q = b half)
    Rows are f32: tiny outputs arise as 1-(values near 1), so rows need
    ~1e-5 ABSOLUTE precision near 1.0 — no 16-bit float has that."""
    key = (kpad, lpos, lneg)
    if key in _PROGRAM_CACHE:
        return _PROGRAM_CACHE[key]

    f32, bf16 = mybir.dt.float32, mybir.dt.bfloat16
    K0, K1, K2 = kpad
    nc = bass.Bass(num_devices=NCORES)
    L = lpos + lneg
    s0, s1, s2 = K0 * 32, K1 * 32, K2 * 128
    rg0_d = nc.declare_dram_parameter("rg0", [128, s0], f32, isOutput=False)
    rg1_d = nc.declare_dram_parameter("rg1", [128, s1], f32, isOutput=False)
    rg2_d = nc.declare_dram_parameter("rg2", [64, s2], f32, isOutput=False)
    scat_d = nc.declare_dram_parameter("scat", [SLOTS, L * NLOC], bf16, isOutput=False)
    base_d = nc.declare_dram_parameter("base", [NLOC, B], f32, isOutput=False)
    u_d = nc.declare_dram_parameter("u", [NLOC, B], f32, isOutput=True)

    with FixedTileContext(nc) as tc:
        with (
            tc.tile_pool(name="sbuf", bufs=1) as pool,
            tc.tile_pool(name="psum", bufs=1, space="PSUM") as psum,
        ):
            # All input DMAs on the sync ring: the model's HWDGE stage is a
            # single shared queue, so issue order IS priority order.  Each
            # group's reduce starts as soon as its own chunk lands.
            rg0 = pool.tile([128, s0], f32)
            nc.sync.dma_start(rg0[:], rg0_d[:])
            rg1 = pool.tile([128, s1], f32)
            nc.sync.dma_start(rg1[:], rg1_d[:])
            rg2 = pool.tile([64, s2], f32)
            nc.sync.dma_start(rg2[:], rg2_d[:])
            sc = pool.tile([SLOTS, L, NLOC], bf16)
            nc.sync.dma_start(
                sc[:], scat_d[:].rearrange("p (l n) -> p l n", l=L))
            base = pool.tile([NLOC, B], f32)
            nc.sync.dma_start(base[:], base_d[:])

            # PE pstate warm stream: dummy matmuls on zeroed data keep the
            # PE clock ramped while the DMAs land, so the real matmuls run
            # at full speed instead of the cold 0.65GHz pstate
            wz = pool.tile([128, 1], bf16)
            nc.gpsimd.memset(wz[:], 0.0)
            wd = pool.tile([128, 512], bf16)
            nc.gpsimd.memset(wd[:], 0.0)
            wp = psum.tile([1, 512], f32, tag="warm")
            for _ in range(10):
                nc.tensor.matmul(wp[:], wz[:], wd[:], start=True, stop=True)

            # w[s,b] = min over literal rows per group (pad rows are 1.0);
            # group 2 is unsplit (64 slots x full B) and reduces straight
            # into its bm partition range
            bm = pool.tile([SLOTS, B], f32)
            w0 = pool.tile([128, 32], f32)
            nc.vector.tensor_reduce(
                out=w0[:], in_=rg0[:].rearrange("p (k b) -> p b k", k=K0),
                axis=mybir.AxisListType.X, op=mybir.AluOpType.min)
            w1 = pool.tile([128, 32], f32)
            nc.vector.tensor_reduce(
                out=w1[:], in_=rg1[:].rearrange("p (k b) -> p b k", k=K1),
                axis=mybir.AxisListType.X, op=mybir.AluOpType.min)

            # reassemble bm[slot, b] with partition-shifted copies, split
            # between Act (group 0, available earliest) and DVE (group 1)
            for q in range(4):
                nc.scalar.copy(bm[0:32, 32 * q:32 * q + 32],
                               w0[32 * q:32 * q + 32, :])
            nc.scalar.copy(bm[32:64, 0:32], w1[0:32, :])
            for q in range(1, 4):
                nc.vector.tensor_scalar(
                    bm[32:64, 32 * q:32 * q + 32],
                    w1[32 * q:32 * q + 32, :],
                    0.0, None, mybir.AluOpType.add)

            # group 2 reduces straight into its bm partition range, last
            # (its DMA is last to land)
            nc.vector.tensor_reduce(
                out=bm[64:128, :],
                in_=rg2[:].rearrange("p (k b) -> p b k", k=K2),
                axis=mybir.AxisListType.X, op=mybir.AluOpType.min)

            # exact bf16 hi/lo split of bm: the one-hot scatter matmuls run
            # at bf16 speed while psum accumulates the f32-accurate sum
            bmh = pool.tile([SLOTS, B], bf16)
            nc.vector.tensor_scalar(
                bmh[:], bm[:], 0.0, None, mybir.AluOpType.add)
            bml = pool.tile([SLOTS, B], bf16)
            nc.vector.tensor_tensor(bml[:], bm[:], bmh[:], mybir.AluOpType.subtract)

            # head scatter: per collision layer, 2 accumulating bf16 matmuls;
            # each sign's layers share a PSUM bank so the cross-layer max is
            # a tensor_reduce
            ptP = psum.tile([NLOC, lpos * B], f32, tag="sp")
            ptN = psum.tile([NLOC, lneg * B], f32, tag="sn")
            for pt, l0, n in ((ptP, 0, lpos), (ptN, lpos, lneg)):
                for l in range(n):
                    nc.tensor.matmul(pt[:, l * B:(l + 1) * B],
                                     sc[:, l0 + l, :], bmh[:],
                                     start=True, stop=False)
                    nc.tensor.matmul(pt[:, l * B:(l + 1) * B],
                                     sc[:, l0 + l, :], bml[:],
                                     start=False, stop=True)

            lb = pool.tile([NLOC, B], f32)
            nc.vector.tensor_reduce(
                out=lb[:], in_=ptP[:].rearrange("p (l b) -> p b l", l=lpos),
                axis=mybir.AxisListType.X, op=mybir.AluOpType.max)
            nmax = pool.tile([NLOC, B], f32)
            nc.vector.tensor_reduce(
                out=nmax[:], in_=ptN[:].rearrange("p (l b) -> p b l", l=lneg),
                axis=mybir.AxisListType.X, op=mybir.AluOpType.max)
            # ub = 1 - nmax on DVE (keeps the tail chain on one engine)
            ub = pool.tile([NLOC, B], f32)
            nc.vector.tensor_scalar(
                ub[:], nmax[:], -1.0, 1.0,
                mybir.AluOpType.mult, mybir.AluOpType.add)

            # u = clip(base, min(lb,ub), max(lb,ub)).  These are pure
            # selections of f32-accurate values (no cancellation), so f32
            # element width is only needed on the inputs; outputs stay f32
            # to keep the final store exact.
            lo = pool.tile([NLOC, B], f32)
            nc.vector.tensor_tensor(lo[:], lb[:], ub[:], mybir.AluOpType.min)
            hi = pool.tile([NLOC, B], f32)
            nc.vector.tensor_tensor(hi[:], lb[:], ub[:], mybir.AluOpType.max)
            mid = pool.tile([NLOC, B], f32)
            nc.vector.tensor_tensor(mid[:], hi[:], base[:], mybir.AluOpType.min)
            u = pool.tile([NLOC, B], f32)
            nc.vector.tensor_tensor(u[:], lo[:], mid[:], mybir.AluOpType.max)
            nc.sync.dma_start(u_d[:], u[:])

    if SPLIT_WAITS:
        split_multi_waits(nc)
    _PROGRAM_CACHE[key] = nc
    return nc


class _Prep:
    """Host-side preprocessing: slot assignment, literal row-index tables,
    one-hot scatter layers.  Everything independent of u1."""

    def __init__(self, preds, goal, atoms, pos_body, neg_body, pos_head, neg_head):
        f32 = np.float32
        import ml_dtypes
        self.bf16 = ml_dtypes.bfloat16
        self.atoms = np.asarray(atoms)
        self.p = preds[:, self.atoms].astype(f32)            # [B, NA]
        self.g = goal[:, self.atoms].astype(f32)
        self.pT = np.ascontiguousarray(self.p.T)             # [NA, B]
        self.gT = np.ascontiguousarray(self.g.T)

        hsum = pos_head + neg_head
        assert np.all(hsum.sum(axis=1) == 1.0), "heads must be one-hot"
        self.h = np.argmax(hsum, axis=1)                     # [C]
        head_is_pos = pos_head[np.arange(C), self.h] == 1.0
        owner = self.h // NLOC

        pos_lists = [np.nonzero(pos_body[c])[0] for c in range(C)]
        neg_lists = [np.nonzero(neg_body[c])[0] for c in range(C)]
        ncnt = np.array([len(pos_lists[c]) + len(neg_lists[c]) for c in range(C)])

        PAD = 4 * NA  # index of the constant-1.0 row in vext

        self.rows1 = []       # per core: [SLOTS, Kmax] row ids for apply-1
        self.rows2 = []       # per core: [SLOTS, Kmax] row ids for apply-2
        lpos_need, lneg_need = 1, 1
        layer_asn = []
        slot_cnt = np.zeros((NCORES, SLOTS), dtype=int)
        Kmax = int(ncnt.max()) + 1
        for i in range(NCORES):
            ci = np.nonzero(owner == i)[0]
            assert len(ci) <= SLOTS, f"core {i} has {len(ci)} constraints"
            ci = ci[np.argsort(-ncnt[ci], kind="stable")]
            slot_cnt[i, : len(ci)] = ncnt[ci] + 1  # +1 head row
            r1 = np.full((SLOTS, Kmax), PAD, dtype=np.int64)
            r2 = np.full((SLOTS, Kmax), PAD, dtype=np.int64)
            for s, c in enumerate(ci):
                rr = np.concatenate([pos_lists[c], NA + neg_lists[c]])
                r1[s, : len(rr)] = rr
                r2[s, : len(rr)] = rr
                hrow = (2 * NA if head_is_pos[c] else 3 * NA) + self.h[c]
                r2[s, len(rr)] = hrow
            self.rows1.append(r1)
            self.rows2.append(r2)

            counts = {}
            asn = []
            for s, c in enumerate(ci):
                key = (self.h[c] % NLOC, bool(head_is_pos[c]))
                l = counts.get(key, 0)
                counts[key] = l + 1
                asn.append((s, key[1], l, key[0]))
                if key[1]:
                    lpos_need = max(lpos_need, l + 1)
                else:
                    lneg_need = max(lneg_need, l + 1)
            layer_asn.append(asn)

        self.lpos, self.lneg = lpos_need, lneg_need
        # per-group row pads (slots sorted desc => group max is at its
        # start).  slot_cnt counts body lits + 1 head row; apply-1 has no
        # head row, so its pads are one smaller.
        def pads(off):
            return tuple(max(int(slot_cnt[:, s].max()) + off, 1)
                         for s in (slice(0, 32), slice(32, 64), slice(64, 128)))
        self.kpad1 = pads(-1)
        self.kpad2 = pads(0)
        self.scat = []
        for i in range(NCORES):
            sc = np.zeros((SLOTS, self.lpos + self.lneg, NLOC), dtype=f32)
            for s, is_pos, l, n in layer_asn[i]:
                li = l if is_pos else self.lpos + l
                sc[s, li, n] = 1.0
            self.scat.append(np.ascontiguousarray(
                sc.reshape(SLOTS, -1)).astype(self.bf16))

    def rg_maps(self, vext: np.ndarray, rows_list, kpad):
        """vext: [4*NA+1, B] value table -> per-core (rg0, rg1, rg2) arrays.
        Group packing: partition = b_chunk * group_width + local_slot."""
        K0, K1, K2 = kpad
        out = []
        for i in range(NCORES):
            gat = vext[rows_list[i]]                     # [SLOTS, Kmax, B]
            g0 = np.ascontiguousarray(
                gat[0:32, :K0, :].reshape(32, K0, 4, 32)
                .transpose(2, 0, 1, 3)).reshape(128, K0 * 32)
            g1 = np.ascontiguousarray(
                gat[32:64, :K1, :].reshape(32, K1, 4, 32)
                .transpose(2, 0, 1, 3)).reshape(128, K1 * 32)
            g2 = np.ascontiguousarray(
                gat[64:128, :K2, :]).reshape(64, K2 * B)
            out.append((g0, g1, g2))
        return out


def kernel(preds, goal, atoms, pos_body, neg_body, pos_head, neg_head):
    preds = np.asarray(preds)
    prep = _Prep(np.asarray(preds, np.float32), np.asarray(goal, np.float32),
                 atoms, np.asarray(pos_body, np.float32),
                 np.asarray(neg_body, np.float32),
                 np.asarray(pos_head, np.float32),
                 np.asarray(neg_head, np.float32))
    core_ids = list(range(NCORES))

    def launch(vext, rows_list, baseT, kpad):
        nc = _build_program(kpad, prep.lpos, prep.lneg)
        rgs = prep.rg_maps(vext, rows_list, kpad)
        in_maps = [{"rg0": rgs[i][0], "rg1": rgs[i][1], "rg2": rgs[i][2],
                    "scat": prep.scat[i],
                    "base": np.ascontiguousarray(
                        baseT[i * NLOC:(i + 1) * NLOC]).astype(np.float32)}
                   for i in range(NCORES)]
        res = run_bass_kernel_spmd(nc, in_maps, core_ids)
        return np.concatenate(
            [res.results[i]["u"] for i in range(NCORES)], axis=0)  # [NA, B]

    f32 = np.float32
    ones = np.ones((1, B), f32)
    pT, gT = prep.pT, prep.gT

    # apply 1: pos lit -> min(p,g); neg lit -> min(1-p,1-g); heads unused
    vext1 = np.concatenate(
        [np.minimum(pT, gT), np.minimum(1.0 - pT, 1.0 - gT),
         np.ones((2 * NA, B), f32), ones], axis=0)
    u1T = launch(vext1, prep.rows1, pT, prep.kpad1)

    # apply 2: pos lit -> max(g,u1); neg lit -> 1-min(g,u1);
    #          head row -> pos: 1-g, neg: g
    vext2 = np.concatenate(
        [np.maximum(gT, u1T), 1.0 - np.minimum(gT, u1T),
         1.0 - gT, gT, ones], axis=0).astype(f32)
    u2T = launch(vext2, prep.rows2, u1T, prep.kpad2)

    out = np.array(preds, dtype=preds.dtype, copy=True)
    out[:, prep.atoms] = u2T.T.astype(preds.dtype)
    return out


# revision 37
# speedup vs baseline: 1.5472x; 1.0082x over previous
"""Trainium2 Bass kernel for nn_ConstraintsModule (fuzzy-logic constraint
propagation).

Algorithm notes
---------------
The reference computes, twice (apply-1 with active=full_body, apply-2 with
active=unsat_head and goal-masked bodies):

    body_rev[b,c,a] = pb[c,a] + v[b,a]*(nb-pb)      -> max over a
    body_min[b,c]   = active[b,c] * (1 - max_a body_rev)
    lb[b,n] = max_c body_min * pos_head[c,n] ; ub = 1 - max_c body_min*neg_head
    u = max(min(lb,ub), min(max(lb,ub), v))

Because bodies are sparse (~4 literals/constraint), heads one-hot, and
``active`` is 0/1 while ``1-W`` is in [0,1], the whole pre-scatter pipeline
collapses into ONE min-reduce over host-gathered literal rows:

    bm[c,b] = active*(1-W) = min(active, 1-W)
            = min over literal slots k of r[c,k,b]

with per-literal row values (pad rows = 1):
    apply-1  pos lit a: min(p[a], g[a])      neg lit a: min(1-p[a], 1-g[a])
    apply-2  pos lit a: max(g[a], u1[a])     neg lit a: 1 - min(g[a], u1[a])
    apply-2  head row : pos head: 1-g[h]     neg head: g[h]
(the act/unsat_head 0/1 factors are exactly the g-masks folded in above).

So the device program per launch is just:
    min-reduce [slots, K, B] -> bm            (DVE, f32 rows: tiny outputs
        arise as 1-(values near 1); rows need ~1e-5 ABSOLUTE precision
        near 1.0, which no 16-bit float has)
    bm -> exact bf16 hi+lo pair; one-hot head-scatter as accumulating
        bf16 matmul pairs per collision layer   (PE, psum f32 exact)
    max across layers, ub = 1-maxN, u = clip(base, min(lb,ub), max(lb,ub))

Sharding: constraints are owned by the core that owns their head atom
(atom range of 128 per core), so the head-scatter and clamp are core-local.
One compiled SPMD program per launch (apply-1 has no head-mask row, so its
row pads are one smaller); the host rebuilds the gathered literal rows from
u1 between launches (pure data layout).  u1 must return in f32: the host
computes 1-u1 from it, so near-1 values need full absolute precision.
"""
import numpy as np

import concourse.bass as bass
import concourse.tile as tile
from concourse import mybir
from concourse.tile import ScopedClock
from concourse.bass_utils import run_bass_kernel_spmd

B = 128
NCOL = 2048
NA = 1024
C = 512
NCORES = 8
SLOTS = 128          # constraint slots per core (padded)
NLOC = 128           # atoms per core


class FixedTileContext(tile.TileContext):
    """Two workarounds for this walrus/NRT combo: (1) skip the tail
    clear_and_free_semaphores — its InstSemClear makes NRT reject the NEFF at
    load, and NRT resets semaphores per execution anyway; (2) multi-wait
    instructions are split afterwards by split_multi_waits()."""

    def _drain_and_barrier(self, tick_clock, wait_clock):
        drain_inst = self.nc.sync.drain()
        wait_clock.add_sem_waits(
            drain_inst.ins, ScopedClock({None: tick_clock.global_clock})
        )
        self.nc.all_engine_barrier()
        assert self.sems is not None
        popped = self.nc._tile_sem_poison_stack.pop()
        assert popped is self._sem_poison


def split_multi_waits(nc: bass.Bass) -> int:
    """walrus here accepts only ONE sync wait per instruction; Tile's
    add_semaphores attaches several.  Hoist all but one wait onto fresh
    same-engine nops placed immediately before the instruction (engine
    program order is preserved, so blocking semantics are identical)."""
    n_split = 0
    for f in nc.m.functions:
        for b in f.blocks:
            new = []
            for ins in b.instructions:
                si = ins.sync_info
                waits = list(si.on_wait) if si and si.on_wait else []
                if len(waits) > 1:
                    for w in waits[:-1]:
                        nop = mybir.InstNoOp(
                            name=f"waitsplit-{n_split}", ins=[], outs=[])
                        n_split += 1
                        nop.engine = ins.engine
                        nop.sync_info = mybir.SyncInfo(on_wait=[w], on_update=[])
                        new.append(nop)
                    ins.sync_info = mybir.SyncInfo(
                        on_wait=[waits[-1]],
                        on_update=list(si.on_update) if si.on_update else [])
                new.append(ins)
            b.instructions = new
    return n_split


_PROGRAM_CACHE = {}
SPLIT_WAITS = True  # set False when running under CoreSim (sim chokes on the
                    # synthesized nops, and doesn't need the split anyway)


def _build_program(kpad: tuple, lpos: int, lneg: int) -> bass.Bass:
    """One SPMD apply phase.  Inputs are per-core; the same program serves
    both applies (rg / scat contents differ per launch).

    Slot groups (slots sorted by literal count desc), each packed to use all
    128 partitions by splitting the batch:
      G0: slots  0:32,  K0 rows, partition = 32*q + s (q = b quarter)
      G1: slots 32:64,  K1 rows, same quartering
      G2: slots 64:128, K2 rows, partition = 64*q + s (q = b half)
    Rows are f32: tiny outputs arise as 1-(values near 1), so rows need
    ~1e-5 ABSOLUTE precision near 1.0 — no 16-bit float has that."""
    key = (kpad, lpos, lneg)
    if key in _PROGRAM_CACHE:
        return _PROGRAM_CACHE[key]

    f32, bf16 = mybir.dt.float32, mybir.dt.bfloat16
    K0, K1, K2 = kpad
    nc = bass.Bass(num_devices=NCORES)
    L = lpos + lneg
    s0, s1, s2 = K0 * 32, K1 * 32, K2 * 128
    rg0_d = nc.declare_dram_parameter("rg0", [128, s0], f32, isOutput=False)
    rg1_d = nc.declare_dram_parameter("rg1", [128, s1], f32, isOutput=False)
    rg2_d = nc.declare_dram_parameter("rg2", [64, s2], f32, isOutput=False)
    scat_d = nc.declare_dram_parameter("scat", [SLOTS, L * NLOC], bf16, isOutput=False)
    base_d = nc.declare_dram_parameter("base", [NLOC, B], f32, isOutput=False)
    u_d = nc.declare_dram_parameter("u", [NLOC, B], f32, isOutput=True)

    with FixedTileContext(nc) as tc:
        with (
            tc.tile_pool(name="sbuf", bufs=1) as pool,
            tc.tile_pool(name="psum", bufs=1, space="PSUM") as psum,
        ):
            # All input DMAs on the sync ring: the model's HWDGE stage is a
            # single shared queue, so issue order IS priority order.  Each
            # group's reduce starts as soon as its own chunk lands.
            rg0 = pool.tile([128, s0], f32)
            nc.sync.dma_start(rg0[:], rg0_d[:])
            rg1 = pool.tile([128, s1], f32)
            nc.sync.dma_start(rg1[:], rg1_d[:])
            rg2 = pool.tile([64, s2], f32)
            nc.sync.dma_start(rg2[:], rg2_d[:])
            sc = pool.tile([SLOTS, L, NLOC], bf16)
            nc.sync.dma_start(
                sc[:], scat_d[:].rearrange("p (l n) -> p l n", l=L))
            base = pool.tile([NLOC, B], f32)
            nc.sync.dma_start(base[:], base_d[:])

            # PE pstate warm stream: dummy matmuls on zeroed data keep the
            # PE clock ramped while the DMAs land, so the real matmuls run
            # at full speed instead of the cold 0.65GHz pstate
            wz = pool.tile([128, 1], bf16)
            nc.gpsimd.memset(wz[:], 0.0)
            wd = pool.tile([128, 512], bf16)
            nc.gpsimd.memset(wd[:], 0.0)
            wp = psum.tile([1, 512], f32, tag="warm")
            for _ in range(8):
                nc.tensor.matmul(wp[:], wz[:], wd[:], start=True, stop=True)

            # w[s,b] = min over literal rows per group (pad rows are 1.0);
            # group 2 is unsplit (64 slots x full B) and reduces straight
            # into its bm partition range
            bm = pool.tile([SLOTS, B], f32)
            w0 = pool.tile([128, 32], f32)
            nc.vector.tensor_reduce(
                out=w0[:], in_=rg0[:].rearrange("p (k b) -> p b k", k=K0),
                axis=mybir.AxisListType.X, op=mybir.AluOpType.min)
            w1 = pool.tile([128, 32], f32)
            nc.vector.tensor_reduce(
                out=w1[:], in_=rg1[:].rearrange("p (k b) -> p b k", k=K1),
                axis=mybir.AxisListType.X, op=mybir.AluOpType.min)

            # reassemble bm[slot, b] with partition-shifted copies, split
            # between Act (group 0, available earliest) and DVE (group 1)
            for q in range(4):
                nc.scalar.copy(bm[0:32, 32 * q:32 * q + 32],
                               w0[32 * q:32 * q + 32, :])
            nc.scalar.copy(bm[32:64, 0:32], w1[0:32, :])
            for q in range(1, 4):
                nc.vector.tensor_scalar(
                    bm[32:64, 32 * q:32 * q + 32],
                    w1[32 * q:32 * q + 32, :],
                    0.0, None, mybir.AluOpType.add)

            # group 2 reduces straight into its bm partition range, last
            # (its DMA is last to land)
            nc.vector.tensor_reduce(
                out=bm[64:128, :],
                in_=rg2[:].rearrange("p (k b) -> p b k", k=K2),
                axis=mybir.AxisListType.X, op=mybir.AluOpType.min)

            # exact bf16 hi/lo split of bm: the one-hot scatter matmuls run
            # at bf16 speed while psum accumulates the f32-accurate sum
            bmh = pool.tile([SLOTS, B], bf16)
            nc.vector.tensor_scalar(
                bmh[:], bm[:], 0.0, None, mybir.AluOpType.add)
            bml = pool.tile([SLOTS, B], bf16)
            nc.vector.tensor_tensor(bml[:], bm[:], bmh[:], mybir.AluOpType.subtract)

            # head scatter: per collision layer, 2 accumulating bf16 matmuls;
            # each sign's layers share a PSUM bank so the cross-layer max is
            # a tensor_reduce
            ptP = psum.tile([NLOC, lpos * B], f32, tag="sp")
            ptN = psum.tile([NLOC, lneg * B], f32, tag="sn")
            for pt, l0, n in ((ptP, 0, lpos), (ptN, lpos, lneg)):
                for l in range(n):
                    nc.tensor.matmul(pt[:, l * B:(l + 1) * B],
                                     sc[:, l0 + l, :], bmh[:],
                                     start=True, stop=False)
                    nc.tensor.matmul(pt[:, l * B:(l + 1) * B],
                                     sc[:, l0 + l, :], bml[:],
                                     start=False, stop=True)

            lb = pool.tile([NLOC, B], f32)
            nc.vector.tensor_reduce(
                out=lb[:], in_=ptP[:].rearrange("p (l b) -> p b l", l=lpos),
                axis=mybir.AxisListType.X, op=mybir.AluOpType.max)
            nmax = pool.tile([NLOC, B], f32)
            nc.vector.tensor_reduce(
                out=nmax[:], in_=ptN[:].rearrange("p (l b) -> p b l", l=lneg),
                axis=mybir.AxisListType.X, op=mybir.AluOpType.max)
            # ub = 1 - nmax on DVE (keeps the tail chain on one engine)
            ub = pool.tile([NLOC, B], f32)
            nc.vector.tensor_scalar(
                ub[:], nmax[:], -1.0, 1.0,
                mybir.AluOpType.mult, mybir.AluOpType.add)

            # u = clip(base, min(lb,ub), max(lb,ub)).  These are pure
            # selections of f32-accurate values (no cancellation), so f32
            # element width is only needed on the inputs; outputs stay f32
            # to keep the final store exact.
            lo = pool.tile([NLOC, B], f32)
            nc.vector.tensor_tensor(lo[:], lb[:], ub[:], mybir.AluOpType.min)
            hi = pool.tile([NLOC, B], f32)
            nc.vector.tensor_tensor(hi[:], lb[:], ub[:], mybir.AluOpType.max)
            mid = pool.tile([NLOC, B], f32)
            nc.vector.tensor_tensor(mid[:], hi[:], base[:], mybir.AluOpType.min)
            u = pool.tile([NLOC, B], f32)
            nc.vector.tensor_tensor(u[:], lo[:], mid[:], mybir.AluOpType.max)
            nc.sync.dma_start(u_d[:], u[:])

    if SPLIT_WAITS:
        split_multi_waits(nc)
    _PROGRAM_CACHE[key] = nc
    return nc


class _Prep:
    """Host-side preprocessing: slot assignment, literal row-index tables,
    one-hot scatter layers.  Everything independent of u1."""

    def __init__(self, preds, goal, atoms, pos_body, neg_body, pos_head, neg_head):
        f32 = np.float32
        import ml_dtypes
        self.bf16 = ml_dtypes.bfloat16
        self.atoms = np.asarray(atoms)
        self.p = preds[:, self.atoms].astype(f32)            # [B, NA]
        self.g = goal[:, self.atoms].astype(f32)
        self.pT = np.ascontiguousarray(self.p.T)             # [NA, B]
        self.gT = np.ascontiguousarray(self.g.T)

        hsum = pos_head + neg_head
        assert np.all(hsum.sum(axis=1) == 1.0), "heads must be one-hot"
        self.h = np.argmax(hsum, axis=1)                     # [C]
        head_is_pos = pos_head[np.arange(C), self.h] == 1.0
        owner = self.h // NLOC

        pos_lists = [np.nonzero(pos_body[c])[0] for c in range(C)]
        neg_lists = [np.nonzero(neg_body[c])[0] for c in range(C)]
        ncnt = np.array([len(pos_lists[c]) + len(neg_lists[c]) for c in range(C)])

        PAD = 4 * NA  # index of the constant-1.0 row in vext

        self.rows1 = []       # per core: [SLOTS, Kmax] row ids for apply-1
        self.rows2 = []       # per core: [SLOTS, Kmax] row ids for apply-2
        lpos_need, lneg_need = 1, 1
        layer_asn = []
        slot_cnt = np.zeros((NCORES, SLOTS), dtype=int)
        Kmax = int(ncnt.max()) + 1
        for i in range(NCORES):
            ci = np.nonzero(owner == i)[0]
            assert len(ci) <= SLOTS, f"core {i} has {len(ci)} constraints"
            ci = ci[np.argsort(-ncnt[ci], kind="stable")]
            slot_cnt[i, : len(ci)] = ncnt[ci] + 1  # +1 head row
            r1 = np.full((SLOTS, Kmax), PAD, dtype=np.int64)
            r2 = np.full((SLOTS, Kmax), PAD, dtype=np.int64)
            for s, c in enumerate(ci):
                rr = np.concatenate([pos_lists[c], NA + neg_lists[c]])
                r1[s, : len(rr)] = rr
                r2[s, : len(rr)] = rr
                hrow = (2 * NA if head_is_pos[c] else 3 * NA) + self.h[c]
                r2[s, len(rr)] = hrow
            self.rows1.append(r1)
            self.rows2.append(r2)

            counts = {}
            asn = []
            for s, c in enumerate(ci):
                key = (self.h[c] % NLOC, bool(head_is_pos[c]))
                l = counts.get(key, 0)
                counts[key] = l + 1
                asn.append((s, key[1], l, key[0]))
                if key[1]:
                    lpos_need = max(lpos_need, l + 1)
                else:
                    lneg_need = max(lneg_need, l + 1)
            layer_asn.append(asn)

        self.lpos, self.lneg = lpos_need, lneg_need
        # per-group row pads (slots sorted desc => group max is at its
        # start).  slot_cnt counts body lits + 1 head row; apply-1 has no
        # head row, so its pads are one smaller.
        def pads(off):
            return tuple(max(int(slot_cnt[:, s].max()) + off, 1)
                         for s in (slice(0, 32), slice(32, 64), slice(64, 128)))
        self.kpad1 = pads(-1)
        self.kpad2 = pads(0)
        self.scat = []
        for i in range(NCORES):
            sc = np.zeros((SLOTS, self.lpos + self.lneg, NLOC), dtype=f32)
            for s, is_pos, l, n in layer_asn[i]:
                li = l if is_pos else self.lpos + l
                sc[s, li, n] = 1.0
            self.scat.append(np.ascontiguousarray(
                sc.reshape(SLOTS, -1)).astype(self.bf16))

    def rg_maps(self, vext: np.ndarray, rows_list, kpad):
        """vext: [4*NA+1, B] value table -> per-core (rg0, rg1, rg2) arrays.
        Group packing: partition = b_chunk * group_width + local_slot."""
        K0, K1, K2 = kpad
        out = []
        for i in range(NCORES):
            gat = vext[rows_list[i]]                     # [SLOTS, Kmax, B]
            g0 = np.ascontiguousarray(
                gat[0:32, :K0, :].reshape(32, K0, 4, 32)
                .transpose(2, 0, 1, 3)).reshape(128, K0 * 32)
            g1 = np.ascontiguousarray(
                gat[32:64, :K1, :].reshape(32, K1, 4, 32)
                .transpose(2, 0, 1, 3)).reshape(128, K1 * 32)
            g2 = np.ascontiguousarray(
                gat[64:128, :K2, :]).reshape(64, K2 * B)
            out.append((g0, g1, g2))
        return out


def kernel(preds, goal, atoms, pos_body, neg_body, pos_head, neg_head):
    preds = np.asarray(preds)
    prep = _Prep(np.asarray(preds, np.float32), np.asarray(goal, np.float32),
                 atoms, np.asarray(pos_body, np.float32),
                 np.asarray(neg_body, np.float32),
                 np.asarray(pos_head, np.float32),
                 np.asarray(neg_head, np.float32))
    core_ids = list(range(NCORES))

    def launch(vext, rows_list, baseT, kpad):
        nc = _build_program(kpad, prep.lpos, prep.lneg)
        rgs = prep.rg_maps(vext, rows_list, kpad)
        in_maps = [{"rg0": rgs[i][0], "rg1": rgs[i][1], "rg2": rgs[i][2],
                    "scat": prep.scat[i],
                    "base": np.ascontiguousarray(
                        baseT[i * NLOC:(i + 1) * NLOC]).astype(np.float32)}
                   for i in range(NCORES)]
        res = run_bass_kernel_spmd(nc, in_maps, core_ids)
        return np.concatenate(
            [res.results[i]["u"] for i in range(NCORES)], axis=0)  # [NA, B]

    f32 = np.float32
    ones = np.ones((1, B), f32)
    pT, gT = prep.pT, prep.gT

    # apply 1: pos lit -> min(p,g); neg lit -> min(1-p,1-g); heads unused
    vext1 = np.concatenate(
        [np.minimum(pT, gT), np.minimum(1.0 - pT, 1.0 - gT),
         np.ones((2 * NA, B), f32), ones], axis=0)
    u1T = launch(vext1, prep.rows1, pT, prep.kpad1)

    # apply 2: pos lit -> max(g,u1); neg lit -> 1-min(g,u1);
    #          head row -> pos: 1-g, neg: g
    vext2 = np.concatenate(
        [np.maximum(gT, u1T), 1.0 - np.minimum(gT, u1T),
         1.0 - gT, gT, ones], axis=0).astype(f32)
    u2T = launch(vext2, prep.rows2, u1T, prep.kpad2)

    out = np.array(preds, dtype=preds.dtype, copy=True)
    out[:, prep.atoms] = u2T.T.astype(preds.dtype)
    return out


# revision 42
# speedup vs baseline: 1.5663x; 1.0124x over previous
"""Trainium2 Bass kernel for nn_ConstraintsModule (fuzzy-logic constraint
propagation).

Algorithm notes
---------------
The reference computes, twice (apply-1 with active=full_body, apply-2 with
active=unsat_head and goal-masked bodies):

    body_rev[b,c,a] = pb[c,a] + v[b,a]*(nb-pb)      -> max over a
    body_min[b,c]   = active[b,c] * (1 - max_a body_rev)
    lb[b,n] = max_c body_min * pos_head[c,n] ; ub = 1 - max_c body_min*neg_head
    u = max(min(lb,ub), min(max(lb,ub), v))

Because bodies are sparse (~4 literals/constraint), heads one-hot, and
``active`` is 0/1 while ``1-W`` is in [0,1], the whole pre-scatter pipeline
collapses into ONE min-reduce over host-gathered literal rows:

    bm[c,b] = active*(1-W) = min(active, 1-W)
            = min over literal slots k of r[c,k,b]

with per-literal row values (pad rows = 1):
    apply-1  pos lit a: min(p[a], g[a])      neg lit a: min(1-p[a], 1-g[a])
    apply-2  pos lit a: max(g[a], u1[a])     neg lit a: 1 - min(g[a], u1[a])
    apply-2  head row : pos head: 1-g[h]     neg head: g[h]
(the act/unsat_head 0/1 factors are exactly the g-masks folded in above).

So the device program per launch is just:
    min-reduce [slots, K, B] -> bm            (DVE, f32 rows: tiny outputs
        arise as 1-(values near 1); rows need ~1e-5 ABSOLUTE precision
        near 1.0, which no 16-bit float has)
    bm -> exact bf16 hi+lo pair; one-hot head-scatter as accumulating
        bf16 matmul pairs per collision layer   (PE, psum f32 exact)
    max across layers, ub = 1-maxN, u = clip(base, min(lb,ub), max(lb,ub))

Sharding: constraints are owned by the core that owns their head atom
(atom range of 128 per core), so the head-scatter and clamp are core-local.
One compiled SPMD program per launch (apply-1 has no head-mask row, so its
row pads are one smaller); the host rebuilds the gathered literal rows from
u1 between launches (pure data layout).  u1 must return in f32: the host
computes 1-u1 from it, so near-1 values need full absolute precision.
"""
import numpy as np

import concourse.bass as bass
import concourse.tile as tile
from concourse import mybir
from concourse.tile import ScopedClock
from concourse.bass_utils import run_bass_kernel_spmd

B = 128
NCOL = 2048
NA = 1024
C = 512
NCORES = 8
SLOTS = 128          # constraint slots per core (padded)
NLOC = 128           # atoms per core


class FixedTileContext(tile.TileContext):
    """Two workarounds for this walrus/NRT combo: (1) skip the tail
    clear_and_free_semaphores — its InstSemClear makes NRT reject the NEFF at
    load, and NRT resets semaphores per execution anyway; (2) multi-wait
    instructions are split afterwards by split_multi_waits()."""

    def _drain_and_barrier(self, tick_clock, wait_clock):
        drain_inst = self.nc.sync.drain()
        wait_clock.add_sem_waits(
            drain_inst.ins, ScopedClock({None: tick_clock.global_clock})
        )
        self.nc.all_engine_barrier()
        assert self.sems is not None
        popped = self.nc._tile_sem_poison_stack.pop()
        assert popped is self._sem_poison


def split_multi_waits(nc: bass.Bass) -> int:
    """walrus here accepts only ONE sync wait per instruction; Tile's
    add_semaphores attaches several.  Hoist all but one wait onto fresh
    same-engine nops placed immediately before the instruction (engine
    program order is preserved, so blocking semantics are identical)."""
    n_split = 0
    for f in nc.m.functions:
        for b in f.blocks:
            new = []
            for ins in b.instructions:
                si = ins.sync_info
                waits = list(si.on_wait) if si and si.on_wait else []
                if len(waits) > 1:
                    for w in waits[:-1]:
                        nop = mybir.InstNoOp(
                            name=f"waitsplit-{n_split}", ins=[], outs=[])
                        n_split += 1
                        nop.engine = ins.engine
                        nop.sync_info = mybir.SyncInfo(on_wait=[w], on_update=[])
                        new.append(nop)
                    ins.sync_info = mybir.SyncInfo(
                        on_wait=[waits[-1]],
                        on_update=list(si.on_update) if si.on_update else [])
                new.append(ins)
            b.instructions = new
    return n_split


_PROGRAM_CACHE = {}
SPLIT_WAITS = True  # set False when running under CoreSim (sim chokes on the
                    # synthesized nops, and doesn't need the split anyway)


def _build_program(kpad: tuple, lpos: int, lneg: int) -> bass.Bass:
    """One SPMD apply phase.  Inputs are per-core; the same program serves
    both applies (rg / scat contents differ per launch).

    Slot groups (slots sorted by literal count desc), each packed to use all
    128 partitions by splitting the batch:
      G0: slots  0:32,  K0 rows, partition = 32*q + s (q = b quarter)
      G1: slots 32:64,  K1 rows, same quartering
      G2: slots 64:128, K2 rows, partition = 64*q + s (q = b half)
    Rows are f32: tiny outputs arise as 1-(values near 1), so rows need
    ~1e-5 ABSOLUTE precision near 1.0 — no 16-bit float has that."""
    key = (kpad, lpos, lneg)
    if key in _PROGRAM_CACHE:
        return _PROGRAM_CACHE[key]

    f32, bf16 = mybir.dt.float32, mybir.dt.bfloat16
    K0, K1, K2 = kpad
    nc = bass.Bass(num_devices=NCORES)
    L = lpos + lneg
    s0, s1, s2 = K0 * 32, K1 * 32, K2 * 128
    rg0_d = nc.declare_dram_parameter("rg0", [128, s0], f32, isOutput=False)
    rg1_d = nc.declare_dram_parameter("rg1", [128, s1], f32, isOutput=False)
    rg2_d = nc.declare_dram_parameter("rg2", [64, s2], f32, isOutput=False)
    scat_d = nc.declare_dram_parameter("scat", [SLOTS, L * NLOC], bf16, isOutput=False)
    base_d = nc.declare_dram_parameter("base", [NLOC, B], f32, isOutput=False)
    u_d = nc.declare_dram_parameter("u", [NLOC, B], f32, isOutput=True)

    with FixedTileContext(nc) as tc:
        with (
            tc.tile_pool(name="sbuf", bufs=1) as pool,
            tc.tile_pool(name="psum", bufs=1, space="PSUM") as psum,
        ):
            # All input DMAs on the sync ring: the model's HWDGE stage is a
            # single shared queue, so issue order IS priority order.  Each
            # group's reduce starts as soon as its own chunk lands.
            rg0 = pool.tile([128, s0], f32)
            nc.sync.dma_start(rg0[:], rg0_d[:])
            rg1 = pool.tile([128, s1], f32)
            nc.sync.dma_start(rg1[:], rg1_d[:])
            rg2 = pool.tile([64, s2], f32)
            nc.sync.dma_start(rg2[:], rg2_d[:])
            sc = pool.tile([SLOTS, L, NLOC], bf16)
            nc.sync.dma_start(
                sc[:], scat_d[:].rearrange("p (l n) -> p l n", l=L))
            base = pool.tile([NLOC, B], f32)
            nc.sync.dma_start(base[:], base_d[:])

            # PE pstate warm stream: dummy matmuls on zeroed data keep the
            # PE clock ramped while the DMAs land, so the real matmuls run
            # at full speed instead of the cold 0.65GHz pstate
            wz = pool.tile([128, 1], bf16)
            nc.gpsimd.memset(wz[:], 0.0)
            wd = pool.tile([128, 512], bf16)
            nc.gpsimd.memset(wd[:], 0.0)
            wp = psum.tile([1, 512], f32, tag="warm")
            for _ in range(8):
                nc.tensor.matmul(wp[:], wz[:], wd[:], start=True, stop=True)

            # w[s,b] = min over literal rows per group (pad rows are 1.0);
            # group 2 is unsplit (64 slots x full B) and reduces straight
            # into its bm partition range
            bm = pool.tile([SLOTS, B], f32)
            w0 = pool.tile([128, 32], f32)
            nc.vector.tensor_reduce(
                out=w0[:], in_=rg0[:].rearrange("p (k b) -> p b k", k=K0),
                axis=mybir.AxisListType.X, op=mybir.AluOpType.min)
            w1 = pool.tile([128, 32], f32)
            nc.vector.tensor_reduce(
                out=w1[:], in_=rg1[:].rearrange("p (k b) -> p b k", k=K1),
                axis=mybir.AxisListType.X, op=mybir.AluOpType.min)

            # reassemble bm[slot, b] with partition-shifted copies, split
            # between Act (group 0, available earliest) and DVE (group 1)
            for q in range(4):
                nc.scalar.copy(bm[0:32, 32 * q:32 * q + 32],
                               w0[32 * q:32 * q + 32, :])
            for q in range(4):
                nc.vector.tensor_scalar(
                    bm[32:64, 32 * q:32 * q + 32],
                    w1[32 * q:32 * q + 32, :],
                    0.0, None, mybir.AluOpType.add)

            # group 2 reduces straight into its bm partition range, last
            # (its DMA is last to land)
            nc.vector.tensor_reduce(
                out=bm[64:128, :],
                in_=rg2[:].rearrange("p (k b) -> p b k", k=K2),
                axis=mybir.AxisListType.X, op=mybir.AluOpType.min)

            # exact bf16 hi/lo split of bm: the one-hot scatter matmuls run
            # at bf16 speed while psum accumulates the f32-accurate sum
            bmh = pool.tile([SLOTS, B], bf16)
            nc.vector.tensor_scalar(
                bmh[:], bm[:], 0.0, None, mybir.AluOpType.add)
            bml = pool.tile([SLOTS, B], bf16)
            nc.vector.tensor_tensor(bml[:], bm[:], bmh[:], mybir.AluOpType.subtract)

            # head scatter: per collision layer, 2 accumulating bf16 matmuls;
            # each sign's layers share a PSUM bank so the cross-layer max is
            # a tensor_reduce
            ptP = psum.tile([NLOC, lpos * B], f32, tag="sp")
            ptN = psum.tile([NLOC, lneg * B], f32, tag="sn")
            for pt, l0, n in ((ptP, 0, lpos), (ptN, lpos, lneg)):
                for l in range(n):
                    nc.tensor.matmul(pt[:, l * B:(l + 1) * B],
                                     sc[:, l0 + l, :], bmh[:],
                                     start=True, stop=False)
                    nc.tensor.matmul(pt[:, l * B:(l + 1) * B],
                                     sc[:, l0 + l, :], bml[:],
                                     start=False, stop=True)

            lb = pool.tile([NLOC, B], f32)
            nc.vector.tensor_reduce(
                out=lb[:], in_=ptP[:].rearrange("p (l b) -> p b l", l=lpos),
                axis=mybir.AxisListType.X, op=mybir.AluOpType.max)
            nmax = pool.tile([NLOC, B], f32)
            nc.vector.tensor_reduce(
                out=nmax[:], in_=ptN[:].rearrange("p (l b) -> p b l", l=lneg),
                axis=mybir.AxisListType.X, op=mybir.AluOpType.max)
            # ub = 1 - nmax on DVE (keeps the tail chain on one engine)
            ub = pool.tile([NLOC, B], f32)
            nc.vector.tensor_scalar(
                ub[:], nmax[:], -1.0, 1.0,
                mybir.AluOpType.mult, mybir.AluOpType.add)

            # u = clip(base, min(lb,ub), max(lb,ub)).  These are pure
            # selections of f32-accurate values (no cancellation), so f32
            # element width is only needed on the inputs; outputs stay f32
            # to keep the final store exact.
            lo = pool.tile([NLOC, B], f32)
            nc.vector.tensor_tensor(lo[:], lb[:], ub[:], mybir.AluOpType.min)
            hi = pool.tile([NLOC, B], f32)
            nc.vector.tensor_tensor(hi[:], lb[:], ub[:], mybir.AluOpType.max)
            mid = pool.tile([NLOC, B], f32)
            nc.vector.tensor_tensor(mid[:], hi[:], base[:], mybir.AluOpType.min)
            u = pool.tile([NLOC, B], f32)
            nc.vector.tensor_tensor(u[:], lo[:], mid[:], mybir.AluOpType.max)
            nc.sync.dma_start(u_d[:], u[:])

    if SPLIT_WAITS:
        split_multi_waits(nc)
    _PROGRAM_CACHE[key] = nc
    return nc


class _Prep:
    """Host-side preprocessing: slot assignment, literal row-index tables,
    one-hot scatter layers.  Everything independent of u1."""

    def __init__(self, preds, goal, atoms, pos_body, neg_body, pos_head, neg_head):
        f32 = np.float32
        import ml_dtypes
        self.bf16 = ml_dtypes.bfloat16
        self.atoms = np.asarray(atoms)
        self.p = preds[:, self.atoms].astype(f32)            # [B, NA]
        self.g = goal[:, self.atoms].astype(f32)
        self.pT = np.ascontiguousarray(self.p.T)             # [NA, B]
        self.gT = np.ascontiguousarray(self.g.T)

        hsum = pos_head + neg_head
        assert np.all(hsum.sum(axis=1) == 1.0), "heads must be one-hot"
        self.h = np.argmax(hsum, axis=1)                     # [C]
        head_is_pos = pos_head[np.arange(C), self.h] == 1.0
        owner = self.h // NLOC

        pos_lists = [np.nonzero(pos_body[c])[0] for c in range(C)]
        neg_lists = [np.nonzero(neg_body[c])[0] for c in range(C)]
        ncnt = np.array([len(pos_lists[c]) + len(neg_lists[c]) for c in range(C)])

        PAD = 4 * NA  # index of the constant-1.0 row in vext

        self.rows1 = []       # per core: [SLOTS, Kmax] row ids for apply-1
        self.rows2 = []       # per core: [SLOTS, Kmax] row ids for apply-2
        lpos_need, lneg_need = 1, 1
        layer_asn = []
        slot_cnt = np.zeros((NCORES, SLOTS), dtype=int)
        Kmax = int(ncnt.max()) + 1
        for i in range(NCORES):
            ci = np.nonzero(owner == i)[0]
            assert len(ci) <= SLOTS, f"core {i} has {len(ci)} constraints"
            ci = ci[np.argsort(-ncnt[ci], kind="stable")]
            slot_cnt[i, : len(ci)] = ncnt[ci] + 1  # +1 head row
            r1 = np.full((SLOTS, Kmax), PAD, dtype=np.int64)
            r2 = np.full((SLOTS, Kmax), PAD, dtype=np.int64)
            for s, c in enumerate(ci):
                rr = np.concatenate([pos_lists[c], NA + neg_lists[c]])
                r1[s, : len(rr)] = rr
                r2[s, : len(rr)] = rr
                hrow = (2 * NA if head_is_pos[c] else 3 * NA) + self.h[c]
                r2[s, len(rr)] = hrow
            self.rows1.append(r1)
            self.rows2.append(r2)

            counts = {}
            asn = []
            for s, c in enumerate(ci):
                key = (self.h[c] % NLOC, bool(head_is_pos[c]))
                l = counts.get(key, 0)
                counts[key] = l + 1
                asn.append((s, key[1], l, key[0]))
                if key[1]:
                    lpos_need = max(lpos_need, l + 1)
                else:
                    lneg_need = max(lneg_need, l + 1)
            layer_asn.append(asn)

        self.lpos, self.lneg = lpos_need, lneg_need
        # per-group row pads (slots sorted desc => group max is at its
        # start).  slot_cnt counts body lits + 1 head row; apply-1 has no
        # head row, so its pads are one smaller.
        def pads(off):
            return tuple(max(int(slot_cnt[:, s].max()) + off, 1)
                         for s in (slice(0, 32), slice(32, 64), slice(64, 128)))
        self.kpad1 = pads(-1)
        self.kpad2 = pads(0)
        self.scat = []
        for i in range(NCORES):
            sc = np.zeros((SLOTS, self.lpos + self.lneg, NLOC), dtype=f32)
            for s, is_pos, l, n in layer_asn[i]:
                li = l if is_pos else self.lpos + l
                sc[s, li, n] = 1.0
            self.scat.append(np.ascontiguousarray(
                sc.reshape(SLOTS, -1)).astype(self.bf16))

    def rg_maps(self, vext: np.ndarray, rows_list, kpad):
        """vext: [4*NA+1, B] value table -> per-core (rg0, rg1, rg2) arrays.
        Group packing: partition = b_chunk * group_width + local_slot."""
        K0, K1, K2 = kpad
        out = []
        for i in range(NCORES):
            gat = vext[rows_list[i]]                     # [SLOTS, Kmax, B]
            g0 = np.ascontiguousarray(
                gat[0:32, :K0, :].reshape(32, K0, 4, 32)
                .transpose(2, 0, 1, 3)).reshape(128, K0 * 32)
            g1 = np.ascontiguousarray(
                gat[32:64, :K1, :].reshape(32, K1, 4, 32)
                .transpose(2, 0, 1, 3)).reshape(128, K1 * 32)
            g2 = np.ascontiguousarray(
                gat[64:128, :K2, :]).reshape(64, K2 * B)
            out.append((g0, g1, g2))
        return out


def kernel(preds, goal, atoms, pos_body, neg_body, pos_head, neg_head):
    preds = np.asarray(preds)
    prep = _Prep(np.asarray(preds, np.float32), np.asarray(goal, np.float32),
                 atoms, np.asarray(pos_body, np.float32),
                 np.asarray(neg_body, np.float32),
                 np.asarray(pos_head, np.float32),
                 np.asarray(neg_head, np.float32))
    core_ids = list(range(NCORES))

    def launch(vext, rows_list, baseT, kpad):
        nc = _build_program(kpad, prep.lpos, prep.lneg)
        rgs = prep.rg_maps(vext, rows_list, kpad)
        in_maps = [{"rg0": rgs[i][0], "rg1": rgs[i][1], "rg2": rgs[i][2],
                    "scat": prep.scat[i],
                    "base": np.ascontiguousarray(
                        baseT[i * NLOC:(i + 1) * NLOC]).astype(np.float32)}
                   for i in range(NCORES)]
        res = run_bass_kernel_spmd(nc, in_maps, core_ids)
        return np.concatenate(
            [res.results[i]["u"] for i in range(NCORES)], axis=0)  # [NA, B]

    f32 = np.float32
    ones = np.ones((1, B), f32)
    pT, gT = prep.pT, prep.gT

    # apply 1: pos lit -> min(p,g); neg lit -> min(1-p,1-g); heads unused
    vext1 = np.concatenate(
        [np.minimum(pT, gT), np.minimum(1.0 - pT, 1.0 - gT),
         np.ones((2 * NA, B), f32), ones], axis=0)
    u1T = launch(vext1, prep.rows1, pT, prep.kpad1)

    # apply 2: pos lit -> max(g,u1); neg lit -> 1-min(g,u1);
    #          head row -> pos: 1-g, neg: g
    vext2 = np.concatenate(
        [np.maximum(gT, u1T), 1.0 - np.minimum(gT, u1T),
         1.0 - gT, gT, ones], axis=0).astype(f32)
    u2T = launch(vext2, prep.rows2, u1T, prep.kpad2)

    out = np.array(preds, dtype=preds.dtype, copy=True)
    out[:, prep.atoms] = u2T.T.astype(preds.dtype)
    return out
